# revision 13
# baseline (speedup 1.0000x reference)
"""Expert-parallel CMoE kernel for 8 Trainium2 NeuronCores (v3).

Sharding (hardcoded for B=8, T=2048, D=1024, F=2048, E=16, C=1024):
  core k owns batch k (token shift, receptance, output) and experts
  {2k, 2k+1} (FFN). Hash routing is int math on token_ids, done on host;
  the resulting permutations ship to the cores as index tensors.

Structure:
  phase A: 4 chunks of 512 tokens. Two big bf16 chunk loads (x / xprev
    views, partition-contiguous "(p a)" layout for cheap descriptors),
    5 wide DVE ops, ONE multi-row indirect scatter into the quarter
    dispatch buffer (a single scatter avoids the WAW completion chain
    Tile inserts between scatters into the same tile), then the quarter
    A2A. Receptance (xr -> HWDGE xbar transpose -> PE matmuls ->
    sigmoid) interleaves per chunk after the dispatch trigger.
  phase C: per expert: XT transposing gather -> FFN1 -> relu^2 -> FFN2
    with ScalarE PSUM evacuation into [P,2,D] tiles, one scatter per
    quarter-capacity combine chunk, 8 combine A2As total so the tail
    collective is small.
  phase D: r loads early, 4 y-gathers, DVE mul, fp32 stores.
Weights/wrt ship pre-shuffled so SBUF loads are partition-contiguous.
"""
import sys

for _p in ("/opt/trn_rl_repo", "/root/.axon_site/_ro/trn_rl_repo"):
    if _p not in sys.path:
        sys.path.append(_p)

import numpy as np
import ml_dtypes

import concourse.bass as bass
import concourse.bacc as bacc
import concourse.mybir as mybir
import concourse.tile as tile
from concourse.bass_utils import run_bass_kernel_spmd

P = 128
B, T, D, F, E = 8, 2048, 1024, 2048, 16
N = B * T
C = max(4, N // E)          # 1024
HASH_PRIME = 5099
NCORES = 8
EPC = E // NCORES           # experts per core = 2
NQ = 4                      # dispatch quarters
QT = T // NQ                # 512 tokens per dispatch chunk
NCK = 8                     # combine chunks (expert parity x capacity quarter)
CH = C // (NCK // EPC)      # 256 slots per combine chunk
BF16 = mybir.dt.bfloat16
F32 = mybir.dt.float32
I16 = mybir.dt.int16
I32 = mybir.dt.int32
nbf16 = ml_dtypes.bfloat16
AF = mybir.ActivationFunctionType

_CACHE = {}


def _r16(v):
    return int(-(-int(v) // 16) * 16)


def _wrap16(a):
    a = np.asarray(a, np.int16)
    w = a.reshape(-1, 16).T.copy()       # j at [j%16, j//16]
    return np.tile(w, (8, 1))            # replicated across 8 Q7 cores


def _route(token_ids):
    tid = np.asarray(token_ids).reshape(N).astype(np.int64)
    e = (tid * HASH_PRIME) % E
    onehot = (e[:, None] == np.arange(E)).astype(np.int64)
    pos = onehot.cumsum(0)[np.arange(N), e] - 1
    keep = pos < C
    return e, pos, keep


def _build_indices(token_ids):
    e, pos, keep = _route(token_ids)
    src = np.arange(N) // T
    dst = e // EPC
    el = e % EPC
    local_t = np.arange(N) % T

    def pack(mask):
        rank = np.zeros(N, np.int64)
        cnt = np.zeros((NCORES, NCORES), np.int64)
        for n in np.nonzero(mask)[0]:
            rank[n] = cnt[src[n], dst[n]]
            cnt[src[n], dst[n]] += 1
        return rank, _r16(max(cnt.max(), 1))

    # ---- dispatch: 4 chunks by local token quarter
    dq = [pack(keep & (local_t // QT == q)) for q in range(NQ)]
    Kq = tuple(k for _, k in dq)
    OFF1 = np.concatenate([[0], np.cumsum([NCORES * k for k in Kq])])
    R1 = int(OFF1[-1])                   # trash row in recv1

    srcQ = np.zeros(N, np.int64)
    for q in range(NQ):
        rank, K = dq[q]
        inq = local_t // QT == q
        srcQ = np.where(inq & keep, dst * K + rank, srcQ)
        srcQ = np.where(inq & ~keep, NCORES * K, srcQ)

    recv_row = np.full((NCORES, EPC * C), R1, np.int64)
    for q in range(NQ):
        rank, K = dq[q]
        for n in np.nonzero(keep & (local_t // QT == q))[0]:
            recv_row[dst[n], el[n] * C + pos[n]] = \
                OFF1[q] + src[n] * K + rank[n]

    # ---- combine: 8 chunks by (expert parity, capacity quarter)
    order = [(eli, qk) for eli in range(EPC) for qk in range(NCK // EPC)]
    comb = {c: pack(keep & (el == c[0]) & (pos // CH == c[1]))
            for c in order}
    K2 = tuple(comb[c][1] for c in order)
    OFF2 = {}
    acc = 0
    for c, k in zip(order, K2):
        OFF2[c] = acc
        acc += NCORES * k
    R2 = acc                             # trash row in recv2

    sl2 = np.zeros((NCORES, EPC, C), np.int64)
    for c, k in zip(order, K2):
        sl2[:, c[0], c[1] * CH:(c[1] + 1) * CH] = NCORES * k
    ygather = np.full(N, R2, np.int64)
    for n in np.nonzero(keep)[0]:
        c = (el[n], pos[n] // CH)
        rank, k = comb[c]
        sl2[dst[n], el[n], pos[n]] = src[n] * k + rank[n]
        ygather[n] = OFF2[c] + dst[n] * k + rank[n]

    per_core = []
    for k in range(NCORES):
        tok = slice(k * T, (k + 1) * T)
        sq = srcQ[tok]
        # dispatch scatter idx: position j = a*128+p <-> token q*512+4p+a
        # (xk tile [p, a] holds token 4p+a within the chunk)
        sd = np.concatenate(
            [_wrap16(sq[q * QT:(q + 1) * QT].reshape(P, 4).T.reshape(QT))
             for q in range(NQ)], axis=1)
        # combine scatter idx: position j = sub*128+p <-> slot qk*256+j
        sc = np.concatenate(
            [_wrap16(sl2[k, c[0], c[1] * CH:(c[1] + 1) * CH])
             for c in order], axis=1)
        per_core.append({
            "sd16": sd,
            "slot16": _wrap16(recv_row[k]),
            "sc16": sc,
            "ygather16": _wrap16(ygather[tok]),
        })
    return (Kq, K2), per_core


def _build_nc(cfg):
    Kq, K2 = cfg
    OFF1 = np.concatenate([[0], np.cumsum([NCORES * k for k in Kq])])
    R1 = int(OFF1[-1])
    order = [(eli, qk) for eli in range(EPC) for qk in range(NCK // EPC)]
    OFF2 = {}
    acc = 0
    for c, k in zip(order, K2):
        OFF2[c] = acc
        acc += NCORES * k
    R2 = acc
    K2d = dict(zip(order, K2))

    nc = bacc.Bacc("TRN2", target_bir_lowering=False, debug=False,
                   num_devices=NCORES)

    x_ext = nc.dram_tensor("x_ext", [T + 1, D], BF16, kind="ExternalInput")
    maa_k = nc.dram_tensor("maa_k", [1, D], BF16, kind="ExternalInput")
    maa_r = nc.dram_tensor("maa_r", [1, D], BF16, kind="ExternalInput")
    wrt = nc.dram_tensor("wrt", [D, D], BF16, kind="ExternalInput")
    wk = nc.dram_tensor("wk", [EPC, D, F], BF16, kind="ExternalInput")
    wv = nc.dram_tensor("wv", [EPC, F, D], BF16, kind="ExternalInput")
    sd16 = nc.dram_tensor("sd16", [P, T // 16], I16, kind="ExternalInput")
    slot16 = nc.dram_tensor("slot16", [P, EPC * C // 16], I16,
                            kind="ExternalInput")
    sc16 = nc.dram_tensor("sc16", [P, EPC * C // 16], I16,
                          kind="ExternalInput")
    ygather16 = nc.dram_tensor("ygather16", [P, T // 16], I16,
                               kind="ExternalInput")
    out = nc.dram_tensor("out", [T, D], F32, kind="ExternalOutput")

    DC = D // P          # 8
    FC = F // P          # 16
    rg = [list(range(NCORES))]

    with tile.TileContext(nc) as tc:
        with (
            tc.tile_pool(name="dram", bufs=1, space="DRAM") as dram,
            tc.tile_pool(name="misc", bufs=1) as misc,
            tc.tile_pool(name="psh", bufs=2, space="PSUM") as psh,
            tc.tile_pool(name="psy", bufs=2, space="PSUM") as psy,
        ):
            disp = [dram.tile([NCORES * Kq[q] + 1, D], BF16, name=f"disp{q}")
                    for q in range(NQ)]
            recv1 = dram.tile([R1 + 1, D], BF16)
            a2 = {c: dram.tile([NCORES * K2d[c] + 1, D], BF16,
                               name=f"a2_{c[0]}_{c[1]}")
                  for c in order}
            recv2 = dram.tile([R2 + 1, D], BF16)
            xr_dram = dram.tile([T, D], BF16)
            r_buf = dram.tile([T, D], BF16)

            zrow = misc.tile([1, D], BF16)
            nc.vector.memzero(zrow[:])
            nc.scalar.dma_start(out=recv1[R1:R1 + 1, :], in_=zrow[:])
            nc.scalar.dma_start(out=recv2[R2:R2 + 1, :], in_=zrow[:])

            # dispatch index on sync (needed first), the rest on scalar
            sD = misc.tile([P, T // 16], I16)
            nc.sync.dma_start(out=sD[:], in_=sd16[:])
            sl16 = misc.tile([P, EPC * C // 16], I16)
            nc.scalar.dma_start(out=sl16[:], in_=slot16[:])
            sC = misc.tile([P, EPC * C // 16], I16)
            nc.scalar.dma_start(out=sC[:], in_=sc16[:])
            yg16 = misc.tile([P, T // 16], I16)
            nc.scalar.dma_start(out=yg16[:], in_=ygather16[:])

            # zero-fill the scatter-add target regions (pad rows are never
            # gathered on the recv side, but written rows need 0 for +=)
            mxrows = max(max(NCORES * k for k in Kq),
                         max(NCORES * k for k in K2)) // P
            zb = misc.tile([P, mxrows, D], BF16)
            nc.vector.memzero(zb[:])
            for q in range(NQ):
                nc.scalar.dma_start(
                    out=disp[q][0:NCORES * Kq[q], :].rearrange(
                        "(a p) d -> p a d", p=P),
                    in_=zb[:, 0:NCORES * Kq[q] // P, :])

            # ---- phase A (token shift) + receptance, 4 chunks of 512
            with (
                tc.tile_pool(name="pa", bufs=2) as pa,
                tc.tile_pool(name="pam", bufs=1) as pam,
                tc.tile_pool(name="prx", bufs=2) as prx,
                tc.tile_pool(name="psr", bufs=1, space="PSUM") as psr,
            ):
                maakb = pam.tile([P, 4, D], BF16)
                maarb = pam.tile([P, 4, D], BF16)
                for n in range(4):
                    nc.scalar.dma_start(out=maakb[:, n, :],
                                        in_=maa_k[:].to_broadcast([P, D]))
                    nc.scalar.dma_start(out=maarb[:, n, :],
                                        in_=maa_r[:].to_broadcast([P, D]))
                # wrt ships pre-shuffled so this is partition-contiguous
                wrt_sb = pam.tile([P, DC, D], BF16)
                nc.scalar.dma_start(out=wrt_sb[:],
                                    in_=wrt.rearrange("(p c) e -> p c e",
                                                      p=P))

                for q in range(NQ):
                    xq = pa.tile([P, 4, D], BF16, tag="xq")
                    nc.sync.dma_start(
                        out=xq[:],
                        in_=x_ext[1 + q * QT:1 + (q + 1) * QT, :].rearrange(
                            "(p a) d -> p a d", p=P))
                    xpq = pa.tile([P, 4, D], BF16, tag="xpq")
                    nc.sync.dma_start(
                        out=xpq[:],
                        in_=x_ext[q * QT:(q + 1) * QT, :].rearrange(
                            "(p a) d -> p a d", p=P))
                    dx = pa.tile([P, 4, D], BF16, tag="dx")
                    nc.vector.tensor_sub(out=dx[:], in0=xpq[:], in1=xq[:])
                    tmp = pa.tile([P, 4, D], BF16, tag="tmp")
                    nc.vector.tensor_mul(out=tmp[:], in0=dx[:], in1=maakb[:])
                    xk = pa.tile([P, 4, D], BF16, tag="xk")
                    nc.vector.tensor_add(out=xk[:], in0=tmp[:], in1=xq[:])
                    nc.gpsimd.dma_scatter_add(
                        out_ap=disp[q][:], in_ap=xk[:],
                        idxs_ap=sD[:, q * 32:(q + 1) * 32],
                        num_idxs=QT, num_idxs_reg=QT, elem_size=D)
                    nc.gpsimd.collective_compute(
                        "AllToAll", mybir.AluOpType.bypass,
                        replica_groups=rg,
                        ins=[disp[q][0:NCORES * Kq[q], :]],
                        outs=[recv1[int(OFF1[q]):int(OFF1[q + 1]), :]])

                    tmp2 = pa.tile([P, 4, D], BF16, tag="tmp")
                    nc.vector.tensor_mul(out=tmp2[:], in0=dx[:], in1=maarb[:])
                    xr = pa.tile([P, 4, D], BF16, tag="xr")
                    nc.vector.tensor_add(out=xr[:], in0=tmp2[:], in1=xq[:])
                    nc.scalar.dma_start(
                        out=xr_dram[q * QT:(q + 1) * QT, :].rearrange(
                            "(p a) d -> p a d", p=P),
                        in_=xr[:])

                    # receptance for this chunk (PE soaks while A2A flies)
                    xrT = prx.tile([P, DC, QT], BF16, tag="xrT")
                    for dc in range(DC):
                        nc.scalar.dma_start_transpose(
                            out=xrT[:, dc, :],
                            in_=xr_dram[q * QT:(q + 1) * QT,
                                        dc * P:(dc + 1) * P])
                    rsb = prx.tile([P, 4, D], BF16, tag="rsb")
                    for tt in range(4):
                        pr0 = psr.tile([P, 512], F32, space="PSUM", tag="pr0")
                        pr1 = psr.tile([P, 512], F32, space="PSUM", tag="pr1")
                        for dc in range(DC):
                            nc.tensor.matmul(
                                out=pr0[:],
                                lhsT=xrT[:, dc, tt * P:(tt + 1) * P],
                                rhs=wrt_sb[:, dc, 0:512],
                                start=(dc == 0), stop=(dc == DC - 1))
                            nc.tensor.matmul(
                                out=pr1[:],
                                lhsT=xrT[:, dc, tt * P:(tt + 1) * P],
                                rhs=wrt_sb[:, dc, 512:1024],
                                start=(dc == 0), stop=(dc == DC - 1))
                        nc.scalar.activation(out=rsb[:, tt, 0:512],
                                             in_=pr0[:], func=AF.Sigmoid)
                        nc.scalar.activation(out=rsb[:, tt, 512:1024],
                                             in_=pr1[:], func=AF.Sigmoid)
                    nc.scalar.dma_start(
                        out=r_buf[q * QT:(q + 1) * QT, :].rearrange(
                            "(a p) d -> p a d", p=P),
                        in_=rsb[:])

            # zero-fill combine scatter targets (needed by ~first FFN output)
            for c in order:
                nc.scalar.dma_start(
                    out=a2[c][0:NCORES * K2d[c], :].rearrange(
                        "(a p) d -> p a d", p=P),
                    in_=zb[:, 0:NCORES * K2d[c] // P, :])

            # ---------------- phase C: expert FFNs
            with (
                tc.tile_pool(name="pwk", bufs=2) as pwk,
                tc.tile_pool(name="pwv", bufs=2) as pwv,
                tc.tile_pool(name="pfx", bufs=2) as pfx,
                tc.tile_pool(name="pfh", bufs=1) as pfh,
                tc.tile_pool(name="pfr", bufs=2) as pfr,
                tc.tile_pool(name="pfy", bufs=2) as pfy,
            ):
                for elp in range(EPC):
                    # weights ship pre-shuffled: partition-contiguous loads
                    wk_sb = pwk.tile([P, DC, F], BF16, tag="wk")
                    nc.sync.dma_start(
                        out=wk_sb[:],
                        in_=wk[elp].rearrange("(p c) f -> p c f", p=P))
                    wv_sb = pwv.tile([P, FC, D], BF16, tag="wv")
                    nc.sync.dma_start(
                        out=wv_sb[:],
                        in_=wv[elp].rearrange("(p c) f -> p c f", p=P))
                    for ck in range(2):
                        XT = pfx.tile([P, DC, 512], BF16, tag="XT")
                        col0 = (elp * C + ck * 512) // 16
                        nc.gpsimd.dma_gather(
                            out_ap=XT[:], in_ap=recv1[:],
                            idxs_ap=sl16[:, col0:col0 + 32],
                            num_idxs=512, num_idxs_reg=512, elem_size=D,
                            transpose=True)
                        ht = pfh.tile([P, FC, 512], BF16, tag="ht")
                        for ft in range(FC):
                            ph = psh.tile([P, 512], F32, space="PSUM",
                                          tag="ph")
                            for dc in range(DC):
                                nc.tensor.matmul(
                                    out=ph[:],
                                    lhsT=wk_sb[:, dc, ft * P:(ft + 1) * P],
                                    rhs=XT[:, dc, :],
                                    start=(dc == 0), stop=(dc == DC - 1))
                            hr = pfr.tile([P, 512], BF16, tag="hr")
                            nc.scalar.activation(out=hr[:], in_=ph[:],
                                                 func=AF.Relu)
                            nc.vector.tensor_mul(out=ht[:, ft, :], in0=hr[:],
                                                 in1=hr[:])
                        for half in range(2):
                            ysb = pfy.tile([P, 2, D], BF16, tag="ysb")
                            for sub in range(2):
                                tt = half * 2 + sub
                                py0 = psy.tile([P, 512], F32, space="PSUM",
                                               tag="py0")
                                py1 = psy.tile([P, 512], F32, space="PSUM",
                                               tag="py1")
                                for fc in range(FC):
                                    nc.tensor.matmul(
                                        out=py0[:],
                                        lhsT=ht[:, fc, tt * P:(tt + 1) * P],
                                        rhs=wv_sb[:, fc, 0:512],
                                        start=(fc == 0), stop=(fc == FC - 1))
                                    nc.tensor.matmul(
                                        out=py1[:],
                                        lhsT=ht[:, fc, tt * P:(tt + 1) * P],
                                        rhs=wv_sb[:, fc, 512:1024],
                                        start=(fc == 0), stop=(fc == FC - 1))
                                nc.scalar.activation(out=ysb[:, sub, 0:512],
                                                     in_=py0[:], func=AF.Copy)
                                nc.scalar.activation(
                                    out=ysb[:, sub, 512:1024],
                                    in_=py1[:], func=AF.Copy)
                            qk = ck * 2 + half
                            cc = (elp, qk)
                            scol = (elp * 4 + qk) * 16
                            nc.gpsimd.dma_scatter_add(
                                out_ap=a2[cc][:], in_ap=ysb[:],
                                idxs_ap=sC[:, scol:scol + 16],
                                num_idxs=CH, num_idxs_reg=CH, elem_size=D)
                            nc.gpsimd.collective_compute(
                                "AllToAll", mybir.AluOpType.bypass,
                                replica_groups=rg,
                                ins=[a2[cc][0:NCORES * K2d[cc], :]],
                                outs=[recv2[OFF2[cc]:
                                            OFF2[cc] + NCORES * K2d[cc], :]])

            # ---------------- phase D: gather own rows, multiply by r
            with (
                tc.tile_pool(name="pdy", bufs=4) as pdy,
                tc.tile_pool(name="pdr", bufs=4) as pdr,
                tc.tile_pool(name="pd", bufs=2) as pd,
            ):
                rws = []
                for ck in range(T // 512):
                    rw = pdr.tile([P, 4, D], BF16, tag="rw")
                    nc.sync.dma_start(
                        out=rw[:],
                        in_=r_buf[ck * 512:(ck + 1) * 512, :].rearrange(
                            "(a p) d -> p a d", p=P))
                    rws.append(rw)
                ygs = []
                for ck in range(T // 512):
                    yg = pdy.tile([P, 4, D], BF16, tag="yg")
                    nc.gpsimd.dma_gather(
                        out_ap=yg[:], in_ap=recv2[:],
                        idxs_ap=yg16[:, ck * 32:(ck + 1) * 32],
                        num_idxs=512, num_idxs_reg=512, elem_size=D,
                        transpose=False)
                    ygs.append(yg)
                for ck in range(T // 512):
                    yo = pd.tile([P, 4, D], F32, tag="yo")
                    nc.vector.tensor_mul(out=yo[:], in0=ygs[ck][:],
                                         in1=rws[ck][:])
                    nc.scalar.dma_start(
                        out=out[ck * 512:(ck + 1) * 512, :].rearrange(
                            "(a p) d -> p a d", p=P),
                        in_=yo[:])

    nc.finalize()
    return nc


def _shuffle_rows(w, nchunks):
    """[R, ...] -> row p*nchunks+c holds original row c*128+p."""
    r = w.shape[0]
    assert r == nchunks * P
    return np.ascontiguousarray(
        w.reshape(nchunks, P, -1).transpose(1, 0, 2).reshape(w.shape))


def _prepare_inputs(x, token_ids, shift_state, time_maa_k, time_maa_r,
                    w_recept, w_key, w_value):
    cfg, idxs = _build_indices(token_ids)
    x = np.asarray(x, np.float32)
    shift = np.asarray(shift_state, np.float32)
    wrt = _shuffle_rows(
        np.ascontiguousarray(np.asarray(w_recept, np.float32).T), D // P
    ).astype(nbf16)
    wkb = np.asarray(w_key, np.float32).astype(nbf16)
    wkb = np.stack([_shuffle_rows(wkb[e], D // P) for e in range(E)])
    wvb = np.asarray(w_value, np.float32).astype(nbf16)
    wvb = np.stack([_shuffle_rows(wvb[e], F // P) for e in range(E)])
    mk = np.asarray(time_maa_k, np.float32)[None, :].astype(nbf16)
    mr = np.asarray(time_maa_r, np.float32)[None, :].astype(nbf16)

    in_maps = []
    for k in range(NCORES):
        x_ext = np.concatenate([shift[k:k + 1], x[k]], axis=0).astype(nbf16)
        in_maps.append({
            "x_ext": np.ascontiguousarray(x_ext),
            "maa_k": mk, "maa_r": mr, "wrt": wrt,
            "wk": np.ascontiguousarray(wkb[EPC * k:EPC * (k + 1)]),
            "wv": np.ascontiguousarray(wvb[EPC * k:EPC * (k + 1)]),
            **idxs[k],
        })
    return cfg, in_maps


def kernel(x, token_ids, shift_state, time_maa_k, time_maa_r,
           w_recept, w_key, w_value, _trace=False):
    cfg, in_maps = _prepare_inputs(x, token_ids, shift_state, time_maa_k,
                                   time_maa_r, w_recept, w_key, w_value)
    if cfg not in _CACHE:
        _CACHE[cfg] = _build_nc(cfg)
    nc = _CACHE[cfg]
    res = run_bass_kernel_spmd(nc, in_maps, core_ids=list(range(NCORES)),
                               trace=_trace)
    kernel.last_result = res
    y = np.stack([res.results[k]["out"] for k in range(NCORES)], axis=0)
    return y.astype(np.float32)


# revision 18
# speedup vs baseline: 1.1871x; 1.1871x over previous
"""Expert-parallel CMoE kernel for 8 Trainium2 NeuronCores (v5).

Sharding (hardcoded for B=8, T=2048, D=1024, F=2048, E=16, C=1024):
  core k owns batch k (token shift, receptance, output) and experts
  {2k, 2k+1} (FFN). Hash routing is int math on token_ids, done on host;
  the resulting permutations ship to the cores as index tensors.

Key scheduling facts this version is built around (from HW traces):
  - Scatters into one tile serialize on each other's DMA completion
    (Tile WAW) -> use ONE dma_scatter_add per chunk into zero-filled
    buffers (pad rows are never gathered, so only written rows need 0).
  - HWDGE xbar dma_start_transpose serializes globally against
    collectives and other DMA modes -> never use it; the xr transpose
    for receptance uses the SWDGE transposing dma_gather (iota idx)
    like the dispatch-side XT gathers.
  - Big partition-contiguous "(p a)" loads keep descriptor counts low.

Pipeline: phase A runs 4 chunks of 512 tokens (2 loads, 5 DVE ops, one
scatter, quarter A2A, then receptance for the chunk: xr store -> gather
-transpose -> PE matmuls -> sigmoid).  FFN per expert with early
single-buffered weight loads; combine is 4 A2As (expert x half).
Phase D gathers y, multiplies by r, stores fp32.
"""
import sys

for _p in ("/opt/trn_rl_repo", "/root/.axon_site/_ro/trn_rl_repo"):
    if _p not in sys.path:
        sys.path.append(_p)

import numpy as np
import ml_dtypes

import concourse.bass as bass
import concourse.bacc as bacc
import concourse.mybir as mybir
import concourse.tile as tile
from concourse.bass_utils import run_bass_kernel_spmd

P = 128
B, T, D, F, E = 8, 2048, 1024, 2048, 16
N = B * T
C = max(4, N // E)          # 1024
HASH_PRIME = 5099
NCORES = 8
EPC = E // NCORES           # experts per core = 2
NQ = 4                      # dispatch quarters
QT = T // NQ                # 512 tokens per dispatch chunk
NCK = 2                     # combine halves per expert
CH = C // NCK               # 512 slots per combine chunk
BF16 = mybir.dt.bfloat16
F32 = mybir.dt.float32
I16 = mybir.dt.int16
I32 = mybir.dt.int32
nbf16 = ml_dtypes.bfloat16
AF = mybir.ActivationFunctionType

_CACHE = {}


def _r16(v):
    return int(-(-int(v) // 16) * 16)


def _wrap16(a):
    a = np.asarray(a, np.int16)
    w = a.reshape(-1, 16).T.copy()       # j at [j%16, j//16]
    return np.tile(w, (8, 1))            # replicated across 8 Q7 cores


def _route(token_ids):
    tid = np.asarray(token_ids).reshape(N).astype(np.int64)
    e = (tid * HASH_PRIME) % E
    onehot = (e[:, None] == np.arange(E)).astype(np.int64)
    pos = onehot.cumsum(0)[np.arange(N), e] - 1
    keep = pos < C
    return e, pos, keep


def _build_indices(token_ids):
    e, pos, keep = _route(token_ids)
    src = np.arange(N) // T
    dst = e // EPC
    el = e % EPC
    local_t = np.arange(N) % T

    def pack(mask):
        rank = np.zeros(N, np.int64)
        cnt = np.zeros((NCORES, NCORES), np.int64)
        for n in np.nonzero(mask)[0]:
            rank[n] = cnt[src[n], dst[n]]
            cnt[src[n], dst[n]] += 1
        return rank, _r16(max(cnt.max(), 1))

    # ---- dispatch: 4 chunks by local token quarter
    dq = [pack(keep & (local_t // QT == q)) for q in range(NQ)]
    Kq = tuple(k for _, k in dq)
    OFF1 = np.concatenate([[0], np.cumsum([NCORES * k for k in Kq])])
    R1 = int(OFF1[-1])                   # trash row in recv1

    srcQ = np.zeros(N, np.int64)
    for q in range(NQ):
        rank, K = dq[q]
        inq = local_t // QT == q
        srcQ = np.where(inq & keep, dst * K + rank, srcQ)
        srcQ = np.where(inq & ~keep, NCORES * K, srcQ)

    recv_row = np.full((NCORES, EPC * C), R1, np.int64)
    for q in range(NQ):
        rank, K = dq[q]
        for n in np.nonzero(keep & (local_t // QT == q))[0]:
            recv_row[dst[n], el[n] * C + pos[n]] = \
                OFF1[q] + src[n] * K + rank[n]

    # ---- combine: 4 chunks by (expert parity, capacity half)
    order = [(eli, ck) for eli in range(EPC) for ck in range(NCK)]
    comb = {c: pack(keep & (el == c[0]) & (pos // CH == c[1]))
            for c in order}
    K2 = tuple(comb[c][1] for c in order)
    OFF2 = {}
    acc = 0
    for c, k in zip(order, K2):
        OFF2[c] = acc
        acc += NCORES * k
    R2 = acc                             # trash row in recv2

    sl2 = np.zeros((NCORES, EPC, C), np.int64)
    for c, k in zip(order, K2):
        sl2[:, c[0], c[1] * CH:(c[1] + 1) * CH] = NCORES * k
    ygather = np.full(N, R2, np.int64)
    for n in np.nonzero(keep)[0]:
        c = (el[n], pos[n] // CH)
        rank, k = comb[c]
        sl2[dst[n], el[n], pos[n]] = src[n] * k + rank[n]
        ygather[n] = OFF2[c] + dst[n] * k + rank[n]

    per_core = []
    for k in range(NCORES):
        tok = slice(k * T, (k + 1) * T)
        sq = srcQ[tok]
        # dispatch scatter idx: position j = a*128+p <-> token q*512+4p+a
        # (xk tile [p, a] holds token 4p+a within the chunk)
        sd = np.concatenate(
            [_wrap16(sq[q * QT:(q + 1) * QT].reshape(P, 4).T.reshape(QT))
             for q in range(NQ)], axis=1)
        # combine scatter idx: position j = tt*128+p <-> slot ck*512+j
        sc = np.concatenate(
            [_wrap16(sl2[k, c[0], c[1] * CH:(c[1] + 1) * CH])
             for c in order], axis=1)
        per_core.append({
            "sd16": sd,
            "slot16": _wrap16(recv_row[k]),
            "sc16": sc,
            "ygather16": _wrap16(ygather[tok]),
        })
    return (Kq, K2), per_core


def _build_nc(cfg):
    Kq, K2 = cfg
    OFF1 = np.concatenate([[0], np.cumsum([NCORES * k for k in Kq])])
    R1 = int(OFF1[-1])
    order = [(eli, ck) for eli in range(EPC) for ck in range(NCK)]
    OFF2 = {}
    acc = 0
    for c, k in zip(order, K2):
        OFF2[c] = acc
        acc += NCORES * k
    R2 = acc
    K2d = dict(zip(order, K2))

    nc = bacc.Bacc("TRN2", target_bir_lowering=False, debug=False,
                   num_devices=NCORES)

    x_ext = nc.dram_tensor("x_ext", [T + 1, D], BF16, kind="ExternalInput")
    maa_k = nc.dram_tensor("maa_k", [1, D], BF16, kind="ExternalInput")
    maa_r = nc.dram_tensor("maa_r", [1, D], BF16, kind="ExternalInput")
    wrt = nc.dram_tensor("wrt", [D, D], BF16, kind="ExternalInput")
    wk = nc.dram_tensor("wk", [EPC, D, F], BF16, kind="ExternalInput")
    wv = nc.dram_tensor("wv", [EPC, F, D], BF16, kind="ExternalInput")
    sd16 = nc.dram_tensor("sd16", [P, T // 16], I16, kind="ExternalInput")
    slot16 = nc.dram_tensor("slot16", [P, EPC * C // 16], I16,
                            kind="ExternalInput")
    sc16 = nc.dram_tensor("sc16", [P, EPC * C // 16], I16,
                          kind="ExternalInput")
    ygather16 = nc.dram_tensor("ygather16", [P, T // 16], I16,
                               kind="ExternalInput")
    iota16 = nc.dram_tensor("iota16", [P, QT // 16], I16,
                            kind="ExternalInput")
    out = nc.dram_tensor("out", [T, D], F32, kind="ExternalOutput")

    DC = D // P          # 8
    FC = F // P          # 16
    rg = [list(range(NCORES))]

    with tile.TileContext(nc) as tc:
        with (
            tc.tile_pool(name="dram", bufs=1, space="DRAM") as dram,
            tc.tile_pool(name="misc", bufs=1) as misc,
            tc.tile_pool(name="pwk", bufs=1) as pwk,
            tc.tile_pool(name="pwv", bufs=1) as pwv,
            tc.tile_pool(name="psh", bufs=2, space="PSUM") as psh,
            tc.tile_pool(name="psy", bufs=2, space="PSUM") as psy,
        ):
            disp = [dram.tile([NCORES * Kq[q] + 1, D], BF16, name=f"disp{q}")
                    for q in range(NQ)]
            recv1 = dram.tile([R1 + 1, D], BF16)
            a2 = {c: dram.tile([NCORES * K2d[c] + 1, D], BF16,
                               name=f"a2_{c[0]}_{c[1]}")
                  for c in order}
            recv2 = dram.tile([R2 + 1, D], BF16)
            xr_q = [dram.tile([QT, D], BF16, name=f"xrq{q}")
                    for q in range(NQ)]
            r_buf = dram.tile([T, D], BF16)

            zrow = misc.tile([1, D], BF16)
            nc.vector.memzero(zrow[:])
            nc.scalar.dma_start(out=recv1[R1:R1 + 1, :], in_=zrow[:])
            nc.scalar.dma_start(out=recv2[R2:R2 + 1, :], in_=zrow[:])

            # dispatch index on sync (needed first), the rest on scalar
            sD = misc.tile([P, T // 16], I16)
            nc.sync.dma_start(out=sD[:], in_=sd16[:])
            sl16 = misc.tile([P, EPC * C // 16], I16)
            nc.scalar.dma_start(out=sl16[:], in_=slot16[:])
            sC = misc.tile([P, EPC * C // 16], I16)
            nc.scalar.dma_start(out=sC[:], in_=sc16[:])
            yg16 = misc.tile([P, T // 16], I16)
            nc.scalar.dma_start(out=yg16[:], in_=ygather16[:])
            io16 = misc.tile([P, QT // 16], I16)
            nc.scalar.dma_start(out=io16[:], in_=iota16[:])

            # zero-fill the scatter-add target regions (pad rows are never
            # gathered on the recv side, but written rows need 0 for +=)
            mxrows = max(max(NCORES * k for k in Kq),
                         max(NCORES * k for k in K2)) // P
            zb = misc.tile([P, mxrows, D], BF16)
            nc.vector.memzero(zb[:])
            for q in range(NQ):
                nc.scalar.dma_start(
                    out=disp[q][0:NCORES * Kq[q], :].rearrange(
                        "(a p) d -> p a d", p=P),
                    in_=zb[:, 0:NCORES * Kq[q] // P, :])

            wk_t = [pwk.tile([P, DC, F], BF16, tag="wk", name=f"wk_t{i}")
                    for i in range(EPC)]
            wv_t = [pwv.tile([P, FC, D], BF16, tag="wv", name=f"wv_t{i}")
                    for i in range(EPC)]

            # ---- phase A (token shift) + receptance, 4 chunks of 512
            with (
                tc.tile_pool(name="pa", bufs=2) as pa,
                tc.tile_pool(name="pam", bufs=1) as pam,
                tc.tile_pool(name="prx", bufs=1) as prx,
                tc.tile_pool(name="psr", bufs=1, space="PSUM") as psr,
            ):
                maakb = pam.tile([P, D], BF16)
                maarb = pam.tile([P, D], BF16)
                nc.scalar.dma_start(out=maakb[:],
                                    in_=maa_k[:].to_broadcast([P, D]))
                nc.scalar.dma_start(out=maarb[:],
                                    in_=maa_r[:].to_broadcast([P, D]))
                # wrt ships pre-shuffled so this is partition-contiguous
                wrt_sb = pam.tile([P, DC, D], BF16)
                nc.scalar.dma_start(out=wrt_sb[:],
                                    in_=wrt.rearrange("(p c) e -> p c e",
                                                      p=P))

                for q in range(NQ):
                    xq = pa.tile([P, 4, D], BF16, tag="xq")
                    nc.sync.dma_start(
                        out=xq[:],
                        in_=x_ext[1 + q * QT:1 + (q + 1) * QT, :].rearrange(
                            "(p a) d -> p a d", p=P))
                    xpq = pa.tile([P, 4, D], BF16, tag="xpq")
                    nc.sync.dma_start(
                        out=xpq[:],
                        in_=x_ext[q * QT:(q + 1) * QT, :].rearrange(
                            "(p a) d -> p a d", p=P))
                    dx = pa.tile([P, 4, D], BF16, tag="dx")
                    nc.vector.tensor_sub(out=dx[:], in0=xpq[:], in1=xq[:])
                    tmp = pa.tile([P, 4, D], BF16, tag="tmp")
                    for n in range(4):
                        nc.vector.tensor_mul(out=tmp[:, n, :],
                                             in0=dx[:, n, :], in1=maakb[:])
                    xk = pa.tile([P, 4, D], BF16, tag="xk")
                    nc.vector.tensor_add(out=xk[:], in0=tmp[:], in1=xq[:])
                    nc.gpsimd.dma_scatter_add(
                        out_ap=disp[q][:], in_ap=xk[:],
                        idxs_ap=sD[:, q * 32:(q + 1) * 32],
                        num_idxs=QT, num_idxs_reg=QT, elem_size=D)
                    nc.gpsimd.collective_compute(
                        "AllToAll", mybir.AluOpType.bypass,
                        replica_groups=rg,
                        ins=[disp[q][0:NCORES * Kq[q], :]],
                        outs=[recv1[int(OFF1[q]):int(OFF1[q + 1]), :]])

                    # xr built in-place in dx (dx is dead after this)
                    for n in range(4):
                        nc.vector.tensor_mul(out=dx[:, n, :],
                                             in0=dx[:, n, :], in1=maarb[:])
                    nc.vector.tensor_add(out=dx[:], in0=dx[:], in1=xq[:])
                    nc.scalar.dma_start(
                        out=xr_q[q][:].rearrange("(p a) d -> p a d", p=P),
                        in_=dx[:])

                    # receptance for this chunk (PE soaks while A2A flies);
                    # transposing SWDGE gather, same path as the XT gathers
                    xrT = prx.tile([P, DC, QT], BF16, tag="xrT")
                    nc.gpsimd.dma_gather(
                        out_ap=xrT[:], in_ap=xr_q[q][:],
                        idxs_ap=io16[:],
                        num_idxs=QT, num_idxs_reg=QT, elem_size=D,
                        transpose=True)
                    rsb = prx.tile([P, 4, D], BF16, tag="rsb")
                    for tt in range(4):
                        pr0 = psr.tile([P, 512], F32, space="PSUM", tag="pr0")
                        pr1 = psr.tile([P, 512], F32, space="PSUM", tag="pr1")
                        for dc in range(DC):
                            nc.tensor.matmul(
                                out=pr0[:],
                                lhsT=xrT[:, dc, tt * P:(tt + 1) * P],
                                rhs=wrt_sb[:, dc, 0:512],
                                start=(dc == 0), stop=(dc == DC - 1))
                            nc.tensor.matmul(
                                out=pr1[:],
                                lhsT=xrT[:, dc, tt * P:(tt + 1) * P],
                                rhs=wrt_sb[:, dc, 512:1024],
                                start=(dc == 0), stop=(dc == DC - 1))
                        nc.scalar.activation(out=rsb[:, tt, 0:512],
                                             in_=pr0[:], func=AF.Sigmoid)
                        nc.scalar.activation(out=rsb[:, tt, 512:1024],
                                             in_=pr1[:], func=AF.Sigmoid)
                    nc.scalar.dma_start(
                        out=r_buf[q * QT:(q + 1) * QT, :].rearrange(
                            "(a p) d -> p a d", p=P),
                        in_=rsb[:])

            # early weight loads for expert 0 (sync queue, after x loads)
            nc.sync.dma_start(out=wk_t[0][:],
                              in_=wk[0].rearrange("(p c) f -> p c f", p=P))
            nc.sync.dma_start(out=wv_t[0][:],
                              in_=wv[0].rearrange("(p c) f -> p c f", p=P))

            # zero-fill combine scatter targets (needed by ~first FFN output)
            for c in order:
                nc.scalar.dma_start(
                    out=a2[c][0:NCORES * K2d[c], :].rearrange(
                        "(a p) d -> p a d", p=P),
                    in_=zb[:, 0:NCORES * K2d[c] // P, :])

            # ---------------- phase C: expert FFNs
            with (
                tc.tile_pool(name="pfx", bufs=2) as pfx,
                tc.tile_pool(name="pfh", bufs=1) as pfh,
                tc.tile_pool(name="pfr", bufs=2) as pfr,
                tc.tile_pool(name="pfy", bufs=2) as pfy,
            ):
                for elp in range(EPC):
                    if elp > 0:
                        nc.sync.dma_start(
                            out=wk_t[elp][:],
                            in_=wk[elp].rearrange("(p c) f -> p c f", p=P))
                        nc.sync.dma_start(
                            out=wv_t[elp][:],
                            in_=wv[elp].rearrange("(p c) f -> p c f", p=P))
                    wk_sb, wv_sb = wk_t[elp], wv_t[elp]
                    for ck in range(NCK):
                        XT = pfx.tile([P, DC, 512], BF16, tag="XT")
                        col0 = (elp * C + ck * CH) // 16
                        nc.gpsimd.dma_gather(
                            out_ap=XT[:], in_ap=recv1[:],
                            idxs_ap=sl16[:, col0:col0 + 32],
                            num_idxs=512, num_idxs_reg=512, elem_size=D,
                            transpose=True)
                        ht = pfh.tile([P, FC, 512], BF16, tag="ht")
                        for ft in range(FC):
                            ph = psh.tile([P, 512], F32, space="PSUM",
                                          tag="ph")
                            for dc in range(DC):
                                nc.tensor.matmul(
                                    out=ph[:],
                                    lhsT=wk_sb[:, dc, ft * P:(ft + 1) * P],
                                    rhs=XT[:, dc, :],
                                    start=(dc == 0), stop=(dc == DC - 1))
                            hr = pfr.tile([P, 512], BF16, tag="hr")
                            nc.scalar.activation(out=hr[:], in_=ph[:],
                                                 func=AF.Relu)
                            nc.vector.tensor_mul(out=ht[:, ft, :], in0=hr[:],
                                                 in1=hr[:])
                        ysb = pfy.tile([P, 4, D], BF16, tag="ysb")
                        for tt in range(4):
                            py0 = psy.tile([P, 512], F32, space="PSUM",
                                           tag="py0")
                            py1 = psy.tile([P, 512], F32, space="PSUM",
                                           tag="py1")
                            for fc in range(FC):
                                nc.tensor.matmul(
                                    out=py0[:],
                                    lhsT=ht[:, fc, tt * P:(tt + 1) * P],
                                    rhs=wv_sb[:, fc, 0:512],
                                    start=(fc == 0), stop=(fc == FC - 1))
                                nc.tensor.matmul(
                                    out=py1[:],
                                    lhsT=ht[:, fc, tt * P:(tt + 1) * P],
                                    rhs=wv_sb[:, fc, 512:1024],
                                    start=(fc == 0), stop=(fc == FC - 1))
                            nc.scalar.activation(out=ysb[:, tt, 0:512],
                                                 in_=py0[:], func=AF.Copy)
                            nc.scalar.activation(out=ysb[:, tt, 512:1024],
                                                 in_=py1[:], func=AF.Copy)
                        cc = (elp, ck)
                        scol = (elp * NCK + ck) * 32
                        nc.gpsimd.dma_scatter_add(
                            out_ap=a2[cc][:], in_ap=ysb[:],
                            idxs_ap=sC[:, scol:scol + 32],
                            num_idxs=CH, num_idxs_reg=CH, elem_size=D)
                        nc.gpsimd.collective_compute(
                            "AllToAll", mybir.AluOpType.bypass,
                            replica_groups=rg,
                            ins=[a2[cc][0:NCORES * K2d[cc], :]],
                            outs=[recv2[OFF2[cc]:OFF2[cc] + NCORES * K2d[cc],
                                        :]])

            # ---------------- phase D: gather own rows, multiply by r
            with (
                tc.tile_pool(name="pdy", bufs=4) as pdy,
                tc.tile_pool(name="pdr", bufs=4) as pdr,
                tc.tile_pool(name="pd", bufs=2) as pd,
            ):
                rws = []
                for ck in range(T // 512):
                    rw = pdr.tile([P, 4, D], BF16, tag="rw")
                    nc.sync.dma_start(
                        out=rw[:],
                        in_=r_buf[ck * 512:(ck + 1) * 512, :].rearrange(
                            "(a p) d -> p a d", p=P))
                    rws.append(rw)
                ygs = []
                for ck in range(T // 512):
                    yg = pdy.tile([P, 4, D], BF16, tag="yg")
                    nc.gpsimd.dma_gather(
                        out_ap=yg[:], in_ap=recv2[:],
                        idxs_ap=yg16[:, ck * 32:(ck + 1) * 32],
                        num_idxs=512, num_idxs_reg=512, elem_size=D,
                        transpose=False)
                    ygs.append(yg)
                for ck in range(T // 512):
                    yo = pd.tile([P, 4, D], F32, tag="yo")
                    nc.vector.tensor_mul(out=yo[:], in0=ygs[ck][:],
                                         in1=rws[ck][:])
                    nc.scalar.dma_start(
                        out=out[ck * 512:(ck + 1) * 512, :].rearrange(
                            "(a p) d -> p a d", p=P),
                        in_=yo[:])

    nc.finalize()
    return nc


def _shuffle_rows(w, nchunks):
    """[R, ...] -> row p*nchunks+c holds original row c*128+p."""
    r = w.shape[0]
    assert r == nchunks * P
    return np.ascontiguousarray(
        w.reshape(nchunks, P, -1).transpose(1, 0, 2).reshape(w.shape))


def _prepare_inputs(x, token_ids, shift_state, time_maa_k, time_maa_r,
                    w_recept, w_key, w_value):
    cfg, idxs = _build_indices(token_ids)
    x = np.asarray(x, np.float32)
    shift = np.asarray(shift_state, np.float32)
    wrt = _shuffle_rows(
        np.ascontiguousarray(np.asarray(w_recept, np.float32).T), D // P
    ).astype(nbf16)
    wkb = np.asarray(w_key, np.float32).astype(nbf16)
    wkb = np.stack([_shuffle_rows(wkb[e], D // P) for e in range(E)])
    wvb = np.asarray(w_value, np.float32).astype(nbf16)
    wvb = np.stack([_shuffle_rows(wvb[e], F // P) for e in range(E)])
    mk = np.asarray(time_maa_k, np.float32)[None, :].astype(nbf16)
    mr = np.asarray(time_maa_r, np.float32)[None, :].astype(nbf16)
    iota = _wrap16(np.arange(QT, dtype=np.int16))

    in_maps = []
    for k in range(NCORES):
        x_ext = np.concatenate([shift[k:k + 1], x[k]], axis=0).astype(nbf16)
        in_maps.append({
            "x_ext": np.ascontiguousarray(x_ext),
            "maa_k": mk, "maa_r": mr, "wrt": wrt,
            "wk": np.ascontiguousarray(wkb[EPC * k:EPC * (k + 1)]),
            "wv": np.ascontiguousarray(wvb[EPC * k:EPC * (k + 1)]),
            "iota16": iota,
            **idxs[k],
        })
    return cfg, in_maps


def kernel(x, token_ids, shift_state, time_maa_k, time_maa_r,
           w_recept, w_key, w_value, _trace=False):
    cfg, in_maps = _prepare_inputs(x, token_ids, shift_state, time_maa_k,
                                   time_maa_r, w_recept, w_key, w_value)
    if cfg not in _CACHE:
        _CACHE[cfg] = _build_nc(cfg)
    nc = _CACHE[cfg]
    res = run_bass_kernel_spmd(nc, in_maps, core_ids=list(range(NCORES)),
                               trace=_trace)
    kernel.last_result = res
    y = np.stack([res.results[k]["out"] for k in range(NCORES)], axis=0)
    return y.astype(np.float32)


# revision 27
# speedup vs baseline: 1.2395x; 1.0442x over previous
"""Expert-parallel CMoE kernel for 8 Trainium2 NeuronCores (v5).

Sharding (hardcoded for B=8, T=2048, D=1024, F=2048, E=16, C=1024):
  core k owns batch k (token shift, receptance, output) and experts
  {2k, 2k+1} (FFN). Hash routing is int math on token_ids, done on host;
  the resulting permutations ship to the cores as index tensors.

Key scheduling facts this version is built around (from HW traces):
  - Scatters into one tile serialize on each other's DMA completion
    (Tile WAW) -> use ONE dma_scatter_add per chunk into zero-filled
    buffers (pad rows are never gathered, so only written rows need 0).
  - HWDGE xbar dma_start_transpose serializes globally against
    collectives and other DMA modes -> never use it; the xr transpose
    for receptance uses the SWDGE transposing dma_gather (iota idx)
    like the dispatch-side XT gathers.
  - Big partition-contiguous "(p a)" loads keep descriptor counts low.

Pipeline: phase A runs 4 chunks of 512 tokens (2 loads, 5 DVE ops, one
scatter, quarter A2A, then receptance for the chunk: xr store -> gather
-transpose -> PE matmuls -> sigmoid).  FFN per expert with early
single-buffered weight loads; combine is 4 A2As (expert x half).
Phase D gathers y, multiplies by r, stores fp32.
"""
import sys

for _p in ("/opt/trn_rl_repo", "/root/.axon_site/_ro/trn_rl_repo"):
    if _p not in sys.path:
        sys.path.append(_p)

import numpy as np
import ml_dtypes

import concourse.bass as bass
import concourse.bacc as bacc
import concourse.mybir as mybir
import concourse.tile as tile
from concourse.tile import add_dep_helper
from concourse.bass_utils import run_bass_kernel_spmd

P = 128
B, T, D, F, E = 8, 2048, 1024, 2048, 16
N = B * T
C = max(4, N // E)          # 1024
HASH_PRIME = 5099
NCORES = 8
EPC = E // NCORES           # experts per core = 2
NQ = 4                      # dispatch quarters
QT = T // NQ                # 512 tokens per dispatch chunk
NCK = 2                     # combine halves per expert
CH = C // NCK               # 512 slots per combine chunk
BF16 = mybir.dt.bfloat16
F32 = mybir.dt.float32
I16 = mybir.dt.int16
I32 = mybir.dt.int32
nbf16 = ml_dtypes.bfloat16
AF = mybir.ActivationFunctionType

_CACHE = {}


def _r16(v):
    return int(-(-int(v) // 16) * 16)


def _wrap16(a):
    a = np.asarray(a, np.int16)
    w = a.reshape(-1, 16).T.copy()       # j at [j%16, j//16]
    return np.tile(w, (8, 1))            # replicated across 8 Q7 cores


def _route(token_ids):
    tid = np.asarray(token_ids).reshape(N).astype(np.int64)
    e = (tid * HASH_PRIME) % E
    onehot = (e[:, None] == np.arange(E)).astype(np.int64)
    pos = onehot.cumsum(0)[np.arange(N), e] - 1
    keep = pos < C
    return e, pos, keep


def _build_indices(token_ids):
    e, pos, keep = _route(token_ids)
    src = np.arange(N) // T
    dst = e // EPC
    el = e % EPC
    local_t = np.arange(N) % T

    def pack(mask):
        rank = np.zeros(N, np.int64)
        cnt = np.zeros((NCORES, NCORES), np.int64)
        for n in np.nonzero(mask)[0]:
            rank[n] = cnt[src[n], dst[n]]
            cnt[src[n], dst[n]] += 1
        return rank, _r16(max(cnt.max(), 1))

    # ---- dispatch: 4 chunks by local token quarter
    dq = [pack(keep & (local_t // QT == q)) for q in range(NQ)]
    Kq = tuple(k for _, k in dq)
    OFF1 = np.concatenate([[0], np.cumsum([NCORES * k for k in Kq])])
    R1 = int(OFF1[-1])                   # trash row in recv1

    srcQ = np.zeros(N, np.int64)
    for q in range(NQ):
        rank, K = dq[q]
        inq = local_t // QT == q
        srcQ = np.where(inq & keep, dst * K + rank, srcQ)
        srcQ = np.where(inq & ~keep, NCORES * K, srcQ)

    recv_row = np.full((NCORES, EPC * C), R1, np.int64)
    for q in range(NQ):
        rank, K = dq[q]
        for n in np.nonzero(keep & (local_t // QT == q))[0]:
            recv_row[dst[n], el[n] * C + pos[n]] = \
                OFF1[q] + src[n] * K + rank[n]

    # ---- combine: 4 chunks by (expert parity, capacity half)
    order = [(eli, ck) for eli in range(EPC) for ck in range(NCK)]
    comb = {c: pack(keep & (el == c[0]) & (pos // CH == c[1]))
            for c in order}
    K2 = tuple(comb[c][1] for c in order)
    OFF2 = {}
    acc = 0
    for c, k in zip(order, K2):
        OFF2[c] = acc
        acc += NCORES * k
    R2 = acc                             # trash row in recv2

    sl2 = np.zeros((NCORES, EPC, C), np.int64)
    for c, k in zip(order, K2):
        sl2[:, c[0], c[1] * CH:(c[1] + 1) * CH] = NCORES * k
    ygather = np.full(N, R2, np.int64)
    for n in np.nonzero(keep)[0]:
        c = (el[n], pos[n] // CH)
        rank, k = comb[c]
        sl2[dst[n], el[n], pos[n]] = src[n] * k + rank[n]
        ygather[n] = OFF2[c] + dst[n] * k + rank[n]

    per_core = []
    for k in range(NCORES):
        tok = slice(k * T, (k + 1) * T)
        sq = srcQ[tok]
        # dispatch scatter idx: position j = a*128+p <-> token q*512+4p+a
        # (xk tile [p, a] holds token 4p+a within the chunk)
        sd = np.concatenate(
            [_wrap16(sq[q * QT:(q + 1) * QT].reshape(P, 4).T.reshape(QT))
             for q in range(NQ)], axis=1)
        # combine scatter idx: position j = tt*128+p <-> slot ck*512+j
        sc = np.concatenate(
            [_wrap16(sl2[k, c[0], c[1] * CH:(c[1] + 1) * CH])
             for c in order], axis=1)
        per_core.append({
            "sd16": sd,
            "slot16": _wrap16(recv_row[k]),
            "sc16": sc,
            "ygather16": _wrap16(ygather[tok]),
        })
    return (Kq, K2), per_core


def _build_nc(cfg):
    Kq, K2 = cfg
    OFF1 = np.concatenate([[0], np.cumsum([NCORES * k for k in Kq])])
    R1 = int(OFF1[-1])
    order = [(eli, ck) for eli in range(EPC) for ck in range(NCK)]
    OFF2 = {}
    acc = 0
    for c, k in zip(order, K2):
        OFF2[c] = acc
        acc += NCORES * k
    R2 = acc
    K2d = dict(zip(order, K2))

    nc = bacc.Bacc("TRN2", target_bir_lowering=False, debug=False,
                   num_devices=NCORES)

    x_ext = nc.dram_tensor("x_ext", [T + 1, D], BF16, kind="ExternalInput")
    maa_k = nc.dram_tensor("maa_k", [1, D], BF16, kind="ExternalInput")
    maa_r = nc.dram_tensor("maa_r", [1, D], BF16, kind="ExternalInput")
    wrt = nc.dram_tensor("wrt", [D, D], BF16, kind="ExternalInput")
    wk = nc.dram_tensor("wk", [EPC, D, F], BF16, kind="ExternalInput")
    wv = nc.dram_tensor("wv", [EPC, F, D], BF16, kind="ExternalInput")
    sd16 = nc.dram_tensor("sd16", [P, T // 16], I16, kind="ExternalInput")
    slot16 = nc.dram_tensor("slot16", [P, EPC * C // 16], I16,
                            kind="ExternalInput")
    sc16 = nc.dram_tensor("sc16", [P, EPC * C // 16], I16,
                          kind="ExternalInput")
    ygather16 = nc.dram_tensor("ygather16", [P, T // 16], I16,
                               kind="ExternalInput")
    iota16 = nc.dram_tensor("iota16", [P, QT // 16], I16,
                            kind="ExternalInput")
    out = nc.dram_tensor("out", [T, D], F32, kind="ExternalOutput")

    DC = D // P          # 8
    FC = F // P          # 16
    rg = [list(range(NCORES))]

    with tile.TileContext(nc) as tc:
        with (
            tc.tile_pool(name="dram", bufs=1, space="DRAM") as dram,
            tc.tile_pool(name="misc", bufs=1) as misc,
            tc.tile_pool(name="pwk", bufs=1) as pwk,
            tc.tile_pool(name="pwv", bufs=1) as pwv,
            tc.tile_pool(name="psh", bufs=2, space="PSUM") as psh,
            tc.tile_pool(name="psy", bufs=2, space="PSUM") as psy,
        ):
            disp = [dram.tile([NCORES * Kq[q] + 1, D], BF16, name=f"disp{q}")
                    for q in range(NQ)]
            recv1 = dram.tile([R1 + 1, D], BF16)
            a2 = {c: dram.tile([NCORES * K2d[c] + 1, D], BF16,
                               name=f"a2_{c[0]}_{c[1]}")
                  for c in order}
            recv2 = dram.tile([R2 + 1, D], BF16)
            r_buf = dram.tile([T, D], BF16)

            zrow = misc.tile([1, D], BF16)
            nc.vector.memzero(zrow[:])
            nc.scalar.dma_start(out=recv1[R1:R1 + 1, :], in_=zrow[:])
            nc.scalar.dma_start(out=recv2[R2:R2 + 1, :], in_=zrow[:])

            # dispatch index on sync (needed first), the rest on scalar
            sD = misc.tile([P, T // 16], I16)
            nc.sync.dma_start(out=sD[:], in_=sd16[:])
            sl16 = misc.tile([P, EPC * C // 16], I16)
            nc.scalar.dma_start(out=sl16[:], in_=slot16[:])
            sC = misc.tile([P, EPC * C // 16], I16)
            nc.scalar.dma_start(out=sC[:], in_=sc16[:])
            yg16 = misc.tile([P, T // 16], I16)
            nc.scalar.dma_start(out=yg16[:], in_=ygather16[:])
            io16 = misc.tile([P, QT // 16], I16)
            nc.scalar.dma_start(out=io16[:], in_=iota16[:])

            # zero-fill the scatter-add target regions (pad rows are never
            # gathered on the recv side, but written rows need 0 for +=)
            mxrows = max(max(NCORES * k for k in Kq),
                         max(NCORES * k for k in K2)) // P
            zb = misc.tile([P, mxrows, D], BF16)
            nc.vector.memzero(zb[:])
            for q in range(NQ):
                nc.scalar.dma_start(
                    out=disp[q][0:NCORES * Kq[q], :].rearrange(
                        "(a p) d -> p a d", p=P),
                    in_=zb[:, 0:NCORES * Kq[q] // P, :])

            wk_t = [pwk.tile([P, DC, F], BF16, tag="wk", name=f"wk_t{i}")
                    for i in range(EPC)]
            wv_t = [pwv.tile([P, FC, D], BF16, tag="wv", name=f"wv_t{i}")
                    for i in range(EPC)]

            # ---- phase A (token shift) + receptance, 4 chunks of 512
            with (
                tc.tile_pool(name="pa", bufs=2) as pa,
                tc.tile_pool(name="pam", bufs=1) as pam,
                tc.tile_pool(name="prx", bufs=2) as prx,
                tc.tile_pool(name="prs", bufs=1) as prs,
                tc.tile_pool(name="psr", bufs=1, space="PSUM") as psr,
            ):
                maakb = pam.tile([P, D], BF16)
                maarb = pam.tile([P, D], BF16)
                nc.scalar.dma_start(out=maakb[:],
                                    in_=maa_k[:].to_broadcast([P, D]))
                nc.scalar.dma_start(out=maarb[:],
                                    in_=maa_r[:].to_broadcast([P, D]))
                # wrt ships pre-shuffled so this is partition-contiguous
                wrt_sb = pam.tile([P, DC, D], BF16)
                nc.scalar.dma_start(out=wrt_sb[:],
                                    in_=wrt.rearrange("(p c) e -> p c e",
                                                      p=P))

                last_scat = None
                for q in range(NQ):
                    xq = pa.tile([P, 4, D], BF16, tag="xq")
                    nc.sync.dma_start(
                        out=xq[:],
                        in_=x_ext[1 + q * QT:1 + (q + 1) * QT, :].rearrange(
                            "(p a) d -> p a d", p=P))
                    # xprev strip for a=0: tokens 4p-1 = x_ext rows q*QT+4p
                    xp0 = pa.tile([P, 1, D], BF16, tag="xp0")
                    nc.sync.dma_start(
                        out=xp0[:],
                        in_=x_ext[q * QT:(q + 1) * QT, :].rearrange(
                            "(p a) d -> p a d", p=P)[:, 0:1, :])
                    # xprev for a=1..3 is xq shifted by one within the tile
                    dx = pa.tile([P, 4, D], BF16, tag="dx")
                    nc.vector.tensor_sub(out=dx[:, 0:1, :], in0=xp0[:],
                                         in1=xq[:, 0:1, :])
                    nc.vector.tensor_sub(out=dx[:, 1:4, :],
                                         in0=xq[:, 0:3, :], in1=xq[:, 1:4, :])
                    tmp = pa.tile([P, 4, D], BF16, tag="tmp")
                    for n in range(4):
                        nc.vector.tensor_mul(out=tmp[:, n, :],
                                             in0=dx[:, n, :], in1=maakb[:])
                    xk = pa.tile([P, 4, D], BF16, tag="xk")
                    nc.vector.tensor_add(out=xk[:], in0=tmp[:], in1=xq[:])
                    last_scat = nc.gpsimd.dma_scatter_add(
                        out_ap=disp[q][:], in_ap=xk[:],
                        idxs_ap=sD[:, q * 32:(q + 1) * 32],
                        num_idxs=QT, num_idxs_reg=QT, elem_size=D)
                    nc.gpsimd.collective_compute(
                        "AllToAll", mybir.AluOpType.bypass,
                        replica_groups=rg,
                        ins=[disp[q][0:NCORES * Kq[q], :]],
                        outs=[recv1[int(OFF1[q]):int(OFF1[q + 1]), :]])

                    # xr built in-place in dx (dx is dead after this)
                    for n in range(4):
                        nc.vector.tensor_mul(out=dx[:, n, :],
                                             in0=dx[:, n, :], in1=maarb[:])
                    nc.vector.tensor_add(out=dx[:], in0=dx[:], in1=xq[:])

                    # receptance for this chunk (PE soaks while A2A flies):
                    # SBUF-source transposing gather straight from the xr
                    # tile -- no DRAM round trip. Layout maps via
                    # tokens_per_rank=128: idx value = a*128+p.
                    xrT = prx.tile([P, DC, QT], BF16, tag="xrT")
                    nc.gpsimd.dma_gather(
                        out_ap=xrT[:], in_ap=dx[:],
                        idxs_ap=io16[:],
                        num_idxs=QT, num_idxs_reg=QT, elem_size=D,
                        transpose=True,
                        sbuf_tokens_per_rank=P,
                        sbuf_free_dim_per_rank=D * 2)
                    rsb = prs.tile([P, 4, D], BF16, tag="rsb")
                    for tt in range(4):
                        pr0 = psr.tile([P, 512], F32, space="PSUM", tag="pr0")
                        pr1 = psr.tile([P, 512], F32, space="PSUM", tag="pr1")
                        for dc in range(DC):
                            nc.tensor.matmul(
                                out=pr0[:],
                                lhsT=xrT[:, dc, tt * P:(tt + 1) * P],
                                rhs=wrt_sb[:, dc, 0:512],
                                start=(dc == 0), stop=(dc == DC - 1))
                            nc.tensor.matmul(
                                out=pr1[:],
                                lhsT=xrT[:, dc, tt * P:(tt + 1) * P],
                                rhs=wrt_sb[:, dc, 512:1024],
                                start=(dc == 0), stop=(dc == DC - 1))
                        nc.scalar.activation(out=rsb[:, tt, 0:512],
                                             in_=pr0[:], func=AF.Sigmoid)
                        nc.scalar.activation(out=rsb[:, tt, 512:1024],
                                             in_=pr1[:], func=AF.Sigmoid)
                    nc.scalar.dma_start(
                        out=r_buf[q * QT:(q + 1) * QT, :].rearrange(
                            "(a p) d -> p a d", p=P),
                        in_=rsb[:])

            # expert-0 weight loads: held back behind the last dispatch
            # scatter so they don't steal HBM from the phase-A window
            wl0 = nc.sync.dma_start(
                out=wk_t[0][:], in_=wk[0].rearrange("(p c) f -> p c f", p=P))
            add_dep_helper(wl0.ins, last_scat.ins,
                           reason="keep wk0 load out of the phase-A window")
            nc.sync.dma_start(out=wv_t[0][:],
                              in_=wv[0].rearrange("(p c) f -> p c f", p=P))

            # zero-fill combine scatter targets (first use is mid-FFN)
            for c in order:
                nc.scalar.dma_start(
                    out=a2[c][0:NCORES * K2d[c], :].rearrange(
                        "(a p) d -> p a d", p=P),
                    in_=zb[:, 0:NCORES * K2d[c] // P, :])

            # ---------------- phase C: expert FFNs
            with (
                tc.tile_pool(name="pfx", bufs=2) as pfx,
                tc.tile_pool(name="pfh", bufs=1) as pfh,
                tc.tile_pool(name="pfr", bufs=2) as pfr,
                tc.tile_pool(name="pfy", bufs=2) as pfy,
            ):
                for elp in range(EPC):
                    if elp > 0:
                        nc.sync.dma_start(
                            out=wk_t[elp][:],
                            in_=wk[elp].rearrange("(p c) f -> p c f", p=P))
                        nc.sync.dma_start(
                            out=wv_t[elp][:],
                            in_=wv[elp].rearrange("(p c) f -> p c f", p=P))
                    wk_sb, wv_sb = wk_t[elp], wv_t[elp]
                    for ck in range(NCK):
                        XT = pfx.tile([P, DC, 512], BF16, tag="XT")
                        col0 = (elp * C + ck * CH) // 16
                        nc.gpsimd.dma_gather(
                            out_ap=XT[:], in_ap=recv1[:],
                            idxs_ap=sl16[:, col0:col0 + 32],
                            num_idxs=512, num_idxs_reg=512, elem_size=D,
                            transpose=True)
                        ht = pfh.tile([P, FC, 512], BF16, tag="ht")
                        for ft in range(FC):
                            ph = psh.tile([P, 512], F32, space="PSUM",
                                          tag="ph")
                            for dc in range(DC):
                                nc.tensor.matmul(
                                    out=ph[:],
                                    lhsT=wk_sb[:, dc, ft * P:(ft + 1) * P],
                                    rhs=XT[:, dc, :],
                                    start=(dc == 0), stop=(dc == DC - 1))
                            hr = pfr.tile([P, 512], BF16, tag="hr")
                            nc.scalar.activation(out=hr[:], in_=ph[:],
                                                 func=AF.Relu)
                            nc.vector.tensor_mul(out=ht[:, ft, :], in0=hr[:],
                                                 in1=hr[:])
                        ysb = pfy.tile([P, 4, D], BF16, tag="ysb")
                        for tt in range(4):
                            py0 = psy.tile([P, 512], F32, space="PSUM",
                                           tag="py0")
                            py1 = psy.tile([P, 512], F32, space="PSUM",
                                           tag="py1")
                            for fc in range(FC):
                                nc.tensor.matmul(
                                    out=py0[:],
                                    lhsT=ht[:, fc, tt * P:(tt + 1) * P],
                                    rhs=wv_sb[:, fc, 0:512],
                                    start=(fc == 0), stop=(fc == FC - 1))
                                nc.tensor.matmul(
                                    out=py1[:],
                                    lhsT=ht[:, fc, tt * P:(tt + 1) * P],
                                    rhs=wv_sb[:, fc, 512:1024],
                                    start=(fc == 0), stop=(fc == FC - 1))
                            nc.scalar.activation(out=ysb[:, tt, 0:512],
                                                 in_=py0[:], func=AF.Copy)
                            nc.scalar.activation(out=ysb[:, tt, 512:1024],
                                                 in_=py1[:], func=AF.Copy)
                        cc = (elp, ck)
                        scol = (elp * NCK + ck) * 32
                        nc.gpsimd.dma_scatter_add(
                            out_ap=a2[cc][:], in_ap=ysb[:],
                            idxs_ap=sC[:, scol:scol + 32],
                            num_idxs=CH, num_idxs_reg=CH, elem_size=D)
                        nc.gpsimd.collective_compute(
                            "AllToAll", mybir.AluOpType.bypass,
                            replica_groups=rg,
                            ins=[a2[cc][0:NCORES * K2d[cc], :]],
                            outs=[recv2[OFF2[cc]:OFF2[cc] + NCORES * K2d[cc],
                                        :]])

            # ---------------- phase D: gather own rows, multiply by r
            with (
                tc.tile_pool(name="pdy", bufs=4) as pdy,
                tc.tile_pool(name="pdr", bufs=4) as pdr,
                tc.tile_pool(name="pd", bufs=2) as pd,
            ):
                rws = []
                for ck in range(T // 512):
                    rw = pdr.tile([P, 4, D], BF16, tag="rw")
                    nc.sync.dma_start(
                        out=rw[:],
                        in_=r_buf[ck * 512:(ck + 1) * 512, :].rearrange(
                            "(a p) d -> p a d", p=P))
                    rws.append(rw)
                ygs = []
                for ck in range(T // 512):
                    yg = pdy.tile([P, 4, D], BF16, tag="yg")
                    nc.gpsimd.dma_gather(
                        out_ap=yg[:], in_ap=recv2[:],
                        idxs_ap=yg16[:, ck * 32:(ck + 1) * 32],
                        num_idxs=512, num_idxs_reg=512, elem_size=D,
                        transpose=False)
                    ygs.append(yg)
                for ck in range(T // 512):
                    yo = pd.tile([P, 4, D], F32, tag="yo")
                    nc.vector.tensor_mul(out=yo[:], in0=ygs[ck][:],
                                         in1=rws[ck][:])
                    nc.scalar.dma_start(
                        out=out[ck * 512:(ck + 1) * 512, :].rearrange(
                            "(a p) d -> p a d", p=P),
                        in_=yo[:])

    nc.finalize()
    return nc


def _shuffle_rows(w, nchunks):
    """[R, ...] -> row p*nchunks+c holds original row c*128+p."""
    r = w.shape[0]
    assert r == nchunks * P
    return np.ascontiguousarray(
        w.reshape(nchunks, P, -1).transpose(1, 0, 2).reshape(w.shape))


def _prepare_inputs(x, token_ids, shift_state, time_maa_k, time_maa_r,
                    w_recept, w_key, w_value):
    cfg, idxs = _build_indices(token_ids)
    x = np.asarray(x, np.float32)
    shift = np.asarray(shift_state, np.float32)
    wrt = _shuffle_rows(
        np.ascontiguousarray(np.asarray(w_recept, np.float32).T), D // P
    ).astype(nbf16)
    wkb = np.asarray(w_key, np.float32).astype(nbf16)
    wkb = np.stack([_shuffle_rows(wkb[e], D // P) for e in range(E)])
    wvb = np.asarray(w_value, np.float32).astype(nbf16)
    wvb = np.stack([_shuffle_rows(wvb[e], F // P) for e in range(E)])
    mk = np.asarray(time_maa_k, np.float32)[None, :].astype(nbf16)
    mr = np.asarray(time_maa_r, np.float32)[None, :].astype(nbf16)
    # SBUF-source gather idx: output position j (= token q*512+j) reads
    # rank j//4 (partition), row j%4 -> idx value = (j%4)*128 + j//4
    j = np.arange(QT, dtype=np.int16)
    iota = _wrap16((j % 4) * P + j // 4)

    in_maps = []
    for k in range(NCORES):
        x_ext = np.concatenate([shift[k:k + 1], x[k]], axis=0).astype(nbf16)
        in_maps.append({
            "x_ext": np.ascontiguousarray(x_ext),
            "maa_k": mk, "maa_r": mr, "wrt": wrt,
            "wk": np.ascontiguousarray(wkb[EPC * k:EPC * (k + 1)]),
            "wv": np.ascontiguousarray(wvb[EPC * k:EPC * (k + 1)]),
            "iota16": iota,
            **idxs[k],
        })
    return cfg, in_maps


def kernel(x, token_ids, shift_state, time_maa_k, time_maa_r,
           w_recept, w_key, w_value, _trace=False):
    cfg, in_maps = _prepare_inputs(x, token_ids, shift_state, time_maa_k,
                                   time_maa_r, w_recept, w_key, w_value)
    if cfg not in _CACHE:
        _CACHE[cfg] = _build_nc(cfg)
    nc = _CACHE[cfg]
    res = run_bass_kernel_spmd(nc, in_maps, core_ids=list(range(NCORES)),
                               trace=_trace)
    kernel.last_result = res
    y = np.stack([res.results[k]["out"] for k in range(NCORES)], axis=0)
    return y.astype(np.float32)


# revision 32
# speedup vs baseline: 1.2591x; 1.0158x over previous
"""Expert-parallel CMoE kernel for 8 Trainium2 NeuronCores (v5).

Sharding (hardcoded for B=8, T=2048, D=1024, F=2048, E=16, C=1024):
  core k owns batch k (token shift, receptance, output) and experts
  {2k, 2k+1} (FFN). Hash routing is int math on token_ids, done on host;
  the resulting permutations ship to the cores as index tensors.

Key scheduling facts this version is built around (from HW traces):
  - Scatters into one tile serialize on each other's DMA completion
    (Tile WAW) -> use ONE dma_scatter_add per chunk into zero-filled
    buffers (pad rows are never gathered, so only written rows need 0).
  - HWDGE xbar dma_start_transpose serializes globally against
    collectives and other DMA modes -> never use it; the xr transpose
    for receptance uses the SWDGE transposing dma_gather (iota idx)
    like the dispatch-side XT gathers.
  - Big partition-contiguous "(p a)" loads keep descriptor counts low.

Pipeline: phase A runs 4 chunks of 512 tokens (2 loads, 5 DVE ops, one
scatter, quarter A2A, then receptance for the chunk: xr store -> gather
-transpose -> PE matmuls -> sigmoid).  FFN per expert with early
single-buffered weight loads; combine is 4 A2As (expert x half).
Phase D gathers y, multiplies by r, stores fp32.
"""
import sys

for _p in ("/opt/trn_rl_repo", "/root/.axon_site/_ro/trn_rl_repo"):
    if _p not in sys.path:
        sys.path.append(_p)

import numpy as np
import ml_dtypes

import concourse.bass as bass
import concourse.bacc as bacc
import concourse.mybir as mybir
import concourse.tile as tile
from concourse.tile import add_dep_helper
from concourse.bass_utils import run_bass_kernel_spmd

P = 128
B, T, D, F, E = 8, 2048, 1024, 2048, 16
N = B * T
C = max(4, N // E)          # 1024
HASH_PRIME = 5099
NCORES = 8
EPC = E // NCORES           # experts per core = 2
NQ = 4                      # dispatch quarters
QT = T // NQ                # 512 tokens per dispatch chunk
NCK = 2                     # combine halves per expert
CH = C // NCK               # 512 slots per combine chunk
BF16 = mybir.dt.bfloat16
F32 = mybir.dt.float32
I16 = mybir.dt.int16
I32 = mybir.dt.int32
nbf16 = ml_dtypes.bfloat16
AF = mybir.ActivationFunctionType

_CACHE = {}


def _r16(v):
    return int(-(-int(v) // 16) * 16)


def _wrap16(a):
    a = np.asarray(a, np.int16)
    w = a.reshape(-1, 16).T.copy()       # j at [j%16, j//16]
    return np.tile(w, (8, 1))            # replicated across 8 Q7 cores


def _route(token_ids):
    tid = np.asarray(token_ids).reshape(N).astype(np.int64)
    e = (tid * HASH_PRIME) % E
    onehot = (e[:, None] == np.arange(E)).astype(np.int64)
    pos = onehot.cumsum(0)[np.arange(N), e] - 1
    keep = pos < C
    return e, pos, keep


def _build_indices(token_ids):
    e, pos, keep = _route(token_ids)
    src = np.arange(N) // T
    dst = e // EPC
    el = e % EPC
    local_t = np.arange(N) % T

    def pack(mask):
        rank = np.zeros(N, np.int64)
        cnt = np.zeros((NCORES, NCORES), np.int64)
        for n in np.nonzero(mask)[0]:
            rank[n] = cnt[src[n], dst[n]]
            cnt[src[n], dst[n]] += 1
        return rank, _r16(max(cnt.max(), 1))

    # ---- dispatch: 4 chunks by local token quarter
    dq = [pack(keep & (local_t // QT == q)) for q in range(NQ)]
    Kq = tuple(k for _, k in dq)
    OFF1 = np.concatenate([[0], np.cumsum([NCORES * k for k in Kq])])
    R1 = int(OFF1[-1])                   # trash row in recv1

    srcQ = np.zeros(N, np.int64)
    for q in range(NQ):
        rank, K = dq[q]
        inq = local_t // QT == q
        srcQ = np.where(inq & keep, dst * K + rank, srcQ)
        srcQ = np.where(inq & ~keep, NCORES * K, srcQ)

    recv_row = np.full((NCORES, EPC * C), R1, np.int64)
    for q in range(NQ):
        rank, K = dq[q]
        for n in np.nonzero(keep & (local_t // QT == q))[0]:
            recv_row[dst[n], el[n] * C + pos[n]] = \
                OFF1[q] + src[n] * K + rank[n]

    # ---- combine: 4 chunks by (expert parity, capacity half)
    order = [(eli, ck) for eli in range(EPC) for ck in range(NCK)]
    comb = {c: pack(keep & (el == c[0]) & (pos // CH == c[1]))
            for c in order}
    K2 = tuple(comb[c][1] for c in order)
    OFF2 = {}
    acc = 0
    for c, k in zip(order, K2):
        OFF2[c] = acc
        acc += NCORES * k
    R2 = acc                             # trash row in recv2

    sl2 = np.zeros((NCORES, EPC, C), np.int64)
    for c, k in zip(order, K2):
        sl2[:, c[0], c[1] * CH:(c[1] + 1) * CH] = NCORES * k
    ygather = np.full(N, R2, np.int64)
    for n in np.nonzero(keep)[0]:
        c = (el[n], pos[n] // CH)
        rank, k = comb[c]
        sl2[dst[n], el[n], pos[n]] = src[n] * k + rank[n]
        ygather[n] = OFF2[c] + dst[n] * k + rank[n]

    per_core = []
    for k in range(NCORES):
        tok = slice(k * T, (k + 1) * T)
        sq = srcQ[tok]
        # dispatch scatter idx: position j = a*128+p <-> token q*512+4p+a
        # (xk tile [p, a] holds token 4p+a within the chunk)
        sd = np.concatenate(
            [_wrap16(sq[q * QT:(q + 1) * QT].reshape(P, 4).T.reshape(QT))
             for q in range(NQ)], axis=1)
        # combine scatter idx: position j = tt*128+p <-> slot ck*512+j
        sc = np.concatenate(
            [_wrap16(sl2[k, c[0], c[1] * CH:(c[1] + 1) * CH])
             for c in order], axis=1)
        per_core.append({
            "sd16": sd,
            "slot16": _wrap16(recv_row[k]),
            "sc16": sc,
            "ygather16": _wrap16(ygather[tok]),
        })
    return (Kq, K2), per_core


def _build_nc(cfg):
    Kq, K2 = cfg
    OFF1 = np.concatenate([[0], np.cumsum([NCORES * k for k in Kq])])
    R1 = int(OFF1[-1])
    order = [(eli, ck) for eli in range(EPC) for ck in range(NCK)]
    OFF2 = {}
    acc = 0
    for c, k in zip(order, K2):
        OFF2[c] = acc
        acc += NCORES * k
    R2 = acc
    K2d = dict(zip(order, K2))

    nc = bacc.Bacc("TRN2", target_bir_lowering=False, debug=False,
                   num_devices=NCORES)

    x_ext = nc.dram_tensor("x_ext", [T + 1, D], BF16, kind="ExternalInput")
    maa_k = nc.dram_tensor("maa_k", [1, D], BF16, kind="ExternalInput")
    maa_r = nc.dram_tensor("maa_r", [1, D], BF16, kind="ExternalInput")
    wrt = nc.dram_tensor("wrt", [D, D], BF16, kind="ExternalInput")
    wk = nc.dram_tensor("wk", [EPC, D, F], BF16, kind="ExternalInput")
    wv = nc.dram_tensor("wv", [EPC, F, D], BF16, kind="ExternalInput")
    sd16 = nc.dram_tensor("sd16", [P, T // 16], I16, kind="ExternalInput")
    slot16 = nc.dram_tensor("slot16", [P, EPC * C // 16], I16,
                            kind="ExternalInput")
    sc16 = nc.dram_tensor("sc16", [P, EPC * C // 16], I16,
                          kind="ExternalInput")
    ygather16 = nc.dram_tensor("ygather16", [P, T // 16], I16,
                               kind="ExternalInput")
    iota16 = nc.dram_tensor("iota16", [P, QT // 16], I16,
                            kind="ExternalInput")
    out = nc.dram_tensor("out", [T, D], F32, kind="ExternalOutput")

    DC = D // P          # 8
    FC = F // P          # 16
    rg = [list(range(NCORES))]

    with tile.TileContext(nc) as tc:
        with (
            tc.tile_pool(name="dram", bufs=1, space="DRAM") as dram,
            tc.tile_pool(name="misc", bufs=1) as misc,
            tc.tile_pool(name="pwk", bufs=1) as pwk,
            tc.tile_pool(name="pwv", bufs=1) as pwv,
            tc.tile_pool(name="psh", bufs=2, space="PSUM") as psh,
            tc.tile_pool(name="psy", bufs=2, space="PSUM") as psy,
        ):
            disp = [dram.tile([NCORES * Kq[q] + 1, D], BF16, name=f"disp{q}")
                    for q in range(NQ)]
            recv1 = dram.tile([R1 + 1, D], BF16)
            a2 = {c: dram.tile([NCORES * K2d[c] + 1, D], BF16,
                               name=f"a2_{c[0]}_{c[1]}")
                  for c in order}
            recv2 = dram.tile([R2 + 1, D], BF16)
            r_buf = dram.tile([T, D], BF16)

            zrow = misc.tile([1, D], BF16)
            nc.vector.memzero(zrow[:])
            nc.scalar.dma_start(out=recv1[R1:R1 + 1, :], in_=zrow[:])
            nc.scalar.dma_start(out=recv2[R2:R2 + 1, :], in_=zrow[:])

            # dispatch index on sync (needed first), the rest on scalar
            sD = misc.tile([P, T // 16], I16)
            nc.sync.dma_start(out=sD[:], in_=sd16[:])
            sl16 = misc.tile([P, EPC * C // 16], I16)
            nc.scalar.dma_start(out=sl16[:], in_=slot16[:])
            sC = misc.tile([P, EPC * C // 16], I16)
            nc.scalar.dma_start(out=sC[:], in_=sc16[:])
            yg16 = misc.tile([P, T // 16], I16)
            nc.scalar.dma_start(out=yg16[:], in_=ygather16[:])
            io16 = misc.tile([P, QT // 16], I16)
            nc.scalar.dma_start(out=io16[:], in_=iota16[:])

            # zero-fill the scatter-add target regions (pad rows are never
            # gathered on the recv side, but written rows need 0 for +=)
            ZR = 4
            zb = misc.tile([P, ZR, D], BF16)
            nc.vector.memzero(zb[:])

            def zero_fill(buf, rows):
                for off in range(0, rows, ZR * P):
                    n = min(ZR * P, rows - off)
                    nc.scalar.dma_start(
                        out=buf[off:off + n, :].rearrange(
                            "(a p) d -> p a d", p=P),
                        in_=zb[:, 0:n // P, :])

            for q in range(NQ):
                zero_fill(disp[q], NCORES * Kq[q])

            wk_t = [pwk.tile([P, DC, F], BF16, tag="wk", name=f"wk_t{i}")
                    for i in range(EPC)]
            wv_t = [pwv.tile([P, FC, D], BF16, tag="wv", name=f"wv_t{i}")
                    for i in range(EPC)]

            # ---- phase A (token shift) + receptance, 4 chunks of 512
            with (
                tc.tile_pool(name="pa", bufs=2) as pa,
                tc.tile_pool(name="pdx", bufs=3) as pdx,
                tc.tile_pool(name="pam", bufs=1) as pam,
                tc.tile_pool(name="prx", bufs=2) as prx,
                tc.tile_pool(name="prs", bufs=1) as prs,
                tc.tile_pool(name="psr", bufs=1, space="PSUM") as psr,
            ):
                maakb = pam.tile([P, D], BF16)
                maarb = pam.tile([P, D], BF16)
                nc.scalar.dma_start(out=maakb[:],
                                    in_=maa_k[:].to_broadcast([P, D]))
                nc.scalar.dma_start(out=maarb[:],
                                    in_=maa_r[:].to_broadcast([P, D]))
                # wrt ships pre-shuffled so this is partition-contiguous
                wrt_sb = pam.tile([P, DC, D], BF16)
                nc.scalar.dma_start(out=wrt_sb[:],
                                    in_=wrt.rearrange("(p c) e -> p c e",
                                                      p=P))

                last_scat = None
                for q in range(NQ):
                    xq = pa.tile([P, 4, D], BF16, tag="xq")
                    nc.sync.dma_start(
                        out=xq[:],
                        in_=x_ext[1 + q * QT:1 + (q + 1) * QT, :].rearrange(
                            "(p a) d -> p a d", p=P))
                    # xprev strip for a=0: tokens 4p-1 = x_ext rows q*QT+4p
                    xp0 = pa.tile([P, 1, D], BF16, tag="xp0")
                    nc.sync.dma_start(
                        out=xp0[:],
                        in_=x_ext[q * QT:(q + 1) * QT, :].rearrange(
                            "(p a) d -> p a d", p=P)[:, 0:1, :])
                    # xprev for a=1..3 is xq shifted by one within the tile
                    dx = pdx.tile([P, 4, D], BF16, tag="dx")
                    nc.vector.tensor_sub(out=dx[:, 0:1, :], in0=xp0[:],
                                         in1=xq[:, 0:1, :])
                    nc.vector.tensor_sub(out=dx[:, 1:4, :],
                                         in0=xq[:, 0:3, :], in1=xq[:, 1:4, :])
                    tmp = pa.tile([P, 4, D], BF16, tag="tmp")
                    for n in range(4):
                        nc.vector.tensor_mul(out=tmp[:, n, :],
                                             in0=dx[:, n, :], in1=maakb[:])
                    xk = pa.tile([P, 4, D], BF16, tag="xk")
                    nc.vector.tensor_add(out=xk[:], in0=tmp[:], in1=xq[:])
                    nc.gpsimd.dma_scatter_add(
                        out_ap=disp[q][:], in_ap=xk[:],
                        idxs_ap=sD[:, q * 32:(q + 1) * 32],
                        num_idxs=QT, num_idxs_reg=QT, elem_size=D)
                    last_trig = nc.gpsimd.collective_compute(
                        "AllToAll", mybir.AluOpType.bypass,
                        replica_groups=rg,
                        ins=[disp[q][0:NCORES * Kq[q], :]],
                        outs=[recv1[int(OFF1[q]):int(OFF1[q + 1]), :]])

                    # xr built in-place in dx (dx is dead after this)
                    for n in range(4):
                        nc.vector.tensor_mul(out=dx[:, n, :],
                                             in0=dx[:, n, :], in1=maarb[:])
                    nc.vector.tensor_add(out=dx[:], in0=dx[:], in1=xq[:])

                    # receptance for this chunk (PE soaks while A2A flies):
                    # SBUF-source transposing gather straight from the xr
                    # tile -- no DRAM round trip. Layout maps via
                    # tokens_per_rank=128: idx value = a*128+p.
                    xrT = prx.tile([P, DC, QT], BF16, tag="xrT")
                    nc.gpsimd.dma_gather(
                        out_ap=xrT[:], in_ap=dx[:],
                        idxs_ap=io16[:],
                        num_idxs=QT, num_idxs_reg=QT, elem_size=D,
                        transpose=True,
                        sbuf_tokens_per_rank=P,
                        sbuf_free_dim_per_rank=D * 2)
                    rsb = prs.tile([P, 4, D], BF16, tag="rsb")
                    for tt in range(4):
                        pr0 = psr.tile([P, 512], F32, space="PSUM", tag="pr0")
                        pr1 = psr.tile([P, 512], F32, space="PSUM", tag="pr1")
                        for dc in range(DC):
                            nc.tensor.matmul(
                                out=pr0[:],
                                lhsT=xrT[:, dc, tt * P:(tt + 1) * P],
                                rhs=wrt_sb[:, dc, 0:512],
                                start=(dc == 0), stop=(dc == DC - 1))
                            nc.tensor.matmul(
                                out=pr1[:],
                                lhsT=xrT[:, dc, tt * P:(tt + 1) * P],
                                rhs=wrt_sb[:, dc, 512:1024],
                                start=(dc == 0), stop=(dc == DC - 1))
                        nc.scalar.activation(out=rsb[:, tt, 0:512],
                                             in_=pr0[:], func=AF.Sigmoid)
                        nc.scalar.activation(out=rsb[:, tt, 512:1024],
                                             in_=pr1[:], func=AF.Sigmoid)
                    nc.scalar.dma_start(
                        out=r_buf[q * QT:(q + 1) * QT, :].rearrange(
                            "(a p) d -> p a d", p=P),
                        in_=rsb[:])

            # expert-0 weight loads: held back behind the last dispatch
            # trigger so they don't steal HBM from the phase-A window
            wl0 = nc.sync.dma_start(
                out=wk_t[0][:], in_=wk[0].rearrange("(p c) f -> p c f", p=P))
            add_dep_helper(wl0.ins, last_trig.ins,
                           reason="keep wk0 load out of the phase-A window")
            nc.sync.dma_start(out=wv_t[0][:],
                              in_=wv[0].rearrange("(p c) f -> p c f", p=P))

            # zero-fill combine scatter targets (first use is mid-FFN)
            for c in order:
                zero_fill(a2[c], NCORES * K2d[c])

            # ---------------- phase C: expert FFNs
            with (
                tc.tile_pool(name="pfx", bufs=2) as pfx,
                tc.tile_pool(name="pfh", bufs=1) as pfh,
                tc.tile_pool(name="pfr", bufs=2) as pfr,
                tc.tile_pool(name="pfy", bufs=2) as pfy,
            ):
                for elp in range(EPC):
                    if elp > 0:
                        nc.sync.dma_start(
                            out=wk_t[elp][:],
                            in_=wk[elp].rearrange("(p c) f -> p c f", p=P))
                        nc.sync.dma_start(
                            out=wv_t[elp][:],
                            in_=wv[elp].rearrange("(p c) f -> p c f", p=P))
                    wk_sb, wv_sb = wk_t[elp], wv_t[elp]
                    for ck in range(NCK):
                        XT = pfx.tile([P, DC, 512], BF16, tag="XT")
                        col0 = (elp * C + ck * CH) // 16
                        nc.gpsimd.dma_gather(
                            out_ap=XT[:], in_ap=recv1[:],
                            idxs_ap=sl16[:, col0:col0 + 32],
                            num_idxs=512, num_idxs_reg=512, elem_size=D,
                            transpose=True)
                        ht = pfh.tile([P, FC, 512], BF16, tag="ht")
                        for ft in range(FC):
                            ph = psh.tile([P, 512], F32, space="PSUM",
                                          tag="ph")
                            for dc in range(DC):
                                nc.tensor.matmul(
                                    out=ph[:],
                                    lhsT=wk_sb[:, dc, ft * P:(ft + 1) * P],
                                    rhs=XT[:, dc, :],
                                    start=(dc == 0), stop=(dc == DC - 1))
                            hr = pfr.tile([P, 512], BF16, tag="hr")
                            nc.scalar.activation(out=hr[:], in_=ph[:],
                                                 func=AF.Relu)
                            nc.vector.tensor_mul(out=ht[:, ft, :], in0=hr[:],
                                                 in1=hr[:])
                        ysb = pfy.tile([P, 4, D], BF16, tag="ysb")
                        for tt in range(4):
                            py0 = psy.tile([P, 512], F32, space="PSUM",
                                           tag="py0")
                            py1 = psy.tile([P, 512], F32, space="PSUM",
                                           tag="py1")
                            for fc in range(FC):
                                nc.tensor.matmul(
                                    out=py0[:],
                                    lhsT=ht[:, fc, tt * P:(tt + 1) * P],
                                    rhs=wv_sb[:, fc, 0:512],
                                    start=(fc == 0), stop=(fc == FC - 1))
                                nc.tensor.matmul(
                                    out=py1[:],
                                    lhsT=ht[:, fc, tt * P:(tt + 1) * P],
                                    rhs=wv_sb[:, fc, 512:1024],
                                    start=(fc == 0), stop=(fc == FC - 1))
                            nc.scalar.activation(out=ysb[:, tt, 0:512],
                                                 in_=py0[:], func=AF.Copy)
                            nc.scalar.activation(out=ysb[:, tt, 512:1024],
                                                 in_=py1[:], func=AF.Copy)
                        cc = (elp, ck)
                        scol = (elp * NCK + ck) * 32
                        nc.gpsimd.dma_scatter_add(
                            out_ap=a2[cc][:], in_ap=ysb[:],
                            idxs_ap=sC[:, scol:scol + 32],
                            num_idxs=CH, num_idxs_reg=CH, elem_size=D)
                        nc.gpsimd.collective_compute(
                            "AllToAll", mybir.AluOpType.bypass,
                            replica_groups=rg,
                            ins=[a2[cc][0:NCORES * K2d[cc], :]],
                            outs=[recv2[OFF2[cc]:OFF2[cc] + NCORES * K2d[cc],
                                        :]])

            # ---------------- phase D: gather own rows, multiply by r
            with (
                tc.tile_pool(name="pdy", bufs=4) as pdy,
                tc.tile_pool(name="pdr", bufs=4) as pdr,
                tc.tile_pool(name="pd", bufs=2) as pd,
            ):
                rws = []
                for ck in range(T // 512):
                    rw = pdr.tile([P, 4, D], BF16, tag="rw")
                    nc.sync.dma_start(
                        out=rw[:],
                        in_=r_buf[ck * 512:(ck + 1) * 512, :].rearrange(
                            "(a p) d -> p a d", p=P))
                    rws.append(rw)
                ygs = []
                for ck in range(T // 512):
                    yg = pdy.tile([P, 4, D], BF16, tag="yg")
                    nc.gpsimd.dma_gather(
                        out_ap=yg[:], in_ap=recv2[:],
                        idxs_ap=yg16[:, ck * 32:(ck + 1) * 32],
                        num_idxs=512, num_idxs_reg=512, elem_size=D,
                        transpose=False)
                    ygs.append(yg)
                for ck in range(T // 512):
                    yo = pd.tile([P, 4, D], F32, tag="yo")
                    nc.vector.tensor_mul(out=yo[:], in0=ygs[ck][:],
                                         in1=rws[ck][:])
                    nc.scalar.dma_start(
                        out=out[ck * 512:(ck + 1) * 512, :].rearrange(
                            "(a p) d -> p a d", p=P),
                        in_=yo[:])

    nc.finalize()
    return nc


def _shuffle_rows(w, nchunks):
    """[R, ...] -> row p*nchunks+c holds original row c*128+p."""
    r = w.shape[0]
    assert r == nchunks * P
    return np.ascontiguousarray(
        w.reshape(nchunks, P, -1).transpose(1, 0, 2).reshape(w.shape))


def _prepare_inputs(x, token_ids, shift_state, time_maa_k, time_maa_r,
                    w_recept, w_key, w_value):
    cfg, idxs = _build_indices(token_ids)
    x = np.asarray(x, np.float32)
    shift = np.asarray(shift_state, np.float32)
    wrt = _shuffle_rows(
        np.ascontiguousarray(np.asarray(w_recept, np.float32).T), D // P
    ).astype(nbf16)
    wkb = np.asarray(w_key, np.float32).astype(nbf16)
    wkb = np.stack([_shuffle_rows(wkb[e], D // P) for e in range(E)])
    wvb = np.asarray(w_value, np.float32).astype(nbf16)
    wvb = np.stack([_shuffle_rows(wvb[e], F // P) for e in range(E)])
    mk = np.asarray(time_maa_k, np.float32)[None, :].astype(nbf16)
    mr = np.asarray(time_maa_r, np.float32)[None, :].astype(nbf16)
    # SBUF-source gather idx: output position j (= token q*512+j) reads
    # rank j//4 (partition), row j%4 -> idx value = (j%4)*128 + j//4
    j = np.arange(QT, dtype=np.int16)
    iota = _wrap16((j % 4) * P + j // 4)

    in_maps = []
    for k in range(NCORES):
        x_ext = np.concatenate([shift[k:k + 1], x[k]], axis=0).astype(nbf16)
        in_maps.append({
            "x_ext": np.ascontiguousarray(x_ext),
            "maa_k": mk, "maa_r": mr, "wrt": wrt,
            "wk": np.ascontiguousarray(wkb[EPC * k:EPC * (k + 1)]),
            "wv": np.ascontiguousarray(wvb[EPC * k:EPC * (k + 1)]),
            "iota16": iota,
            **idxs[k],
        })
    return cfg, in_maps


def kernel(x, token_ids, shift_state, time_maa_k, time_maa_r,
           w_recept, w_key, w_value, _trace=False):
    cfg, in_maps = _prepare_inputs(x, token_ids, shift_state, time_maa_k,
                                   time_maa_r, w_recept, w_key, w_value)
    if cfg not in _CACHE:
        _CACHE[cfg] = _build_nc(cfg)
    nc = _CACHE[cfg]
    res = run_bass_kernel_spmd(nc, in_maps, core_ids=list(range(NCORES)),
                               trace=_trace)
    kernel.last_result = res
    y = np.stack([res.results[k]["out"] for k in range(NCORES)], axis=0)
    return y.astype(np.float32)


# revision 36
# speedup vs baseline: 1.2723x; 1.0105x over previous
"""Expert-parallel CMoE kernel for 8 Trainium2 NeuronCores (v5).

Sharding (hardcoded for B=8, T=2048, D=1024, F=2048, E=16, C=1024):
  core k owns batch k (token shift, receptance, output) and experts
  {2k, 2k+1} (FFN). Hash routing is int math on token_ids, done on host;
  the resulting permutations ship to the cores as index tensors.

Key scheduling facts this version is built around (from HW traces):
  - Scatters into one tile serialize on each other's DMA completion
    (Tile WAW) -> use ONE dma_scatter_add per chunk into zero-filled
    buffers (pad rows are never gathered, so only written rows need 0).
  - HWDGE xbar dma_start_transpose serializes globally against
    collectives and other DMA modes -> never use it; the xr transpose
    for receptance uses the SWDGE transposing dma_gather (iota idx)
    like the dispatch-side XT gathers.
  - Big partition-contiguous "(p a)" loads keep descriptor counts low.

Pipeline: phase A runs 4 chunks of 512 tokens (2 loads, 5 DVE ops, one
scatter, quarter A2A, then receptance for the chunk: xr store -> gather
-transpose -> PE matmuls -> sigmoid).  FFN per expert with early
single-buffered weight loads; combine is 4 A2As (expert x half).
Phase D gathers y, multiplies by r, stores fp32.
"""
import sys

for _p in ("/opt/trn_rl_repo", "/root/.axon_site/_ro/trn_rl_repo"):
    if _p not in sys.path:
        sys.path.append(_p)

import numpy as np
import ml_dtypes

import concourse.bass as bass
import concourse.bacc as bacc
import concourse.mybir as mybir
import concourse.tile as tile
from concourse.tile import add_dep_helper
from concourse.bass_utils import run_bass_kernel_spmd

P = 128
B, T, D, F, E = 8, 2048, 1024, 2048, 16
N = B * T
C = max(4, N // E)          # 1024
HASH_PRIME = 5099
NCORES = 8
EPC = E // NCORES           # experts per core = 2
NQ = 4                      # dispatch quarters
QT = T // NQ                # 512 tokens per dispatch chunk
NCK = 2                     # combine halves per expert
CH = C // NCK               # 512 slots per combine chunk
BF16 = mybir.dt.bfloat16
F32 = mybir.dt.float32
I16 = mybir.dt.int16
I32 = mybir.dt.int32
nbf16 = ml_dtypes.bfloat16
AF = mybir.ActivationFunctionType

_CACHE = {}


def _r16(v):
    return int(-(-int(v) // 16) * 16)


def _wrap16(a):
    a = np.asarray(a, np.int16)
    w = a.reshape(-1, 16).T.copy()       # j at [j%16, j//16]
    return np.tile(w, (8, 1))            # replicated across 8 Q7 cores


def _route(token_ids):
    tid = np.asarray(token_ids).reshape(N).astype(np.int64)
    e = (tid * HASH_PRIME) % E
    onehot = (e[:, None] == np.arange(E)).astype(np.int64)
    pos = onehot.cumsum(0)[np.arange(N), e] - 1
    keep = pos < C
    return e, pos, keep


def _build_indices(token_ids):
    e, pos, keep = _route(token_ids)
    src = np.arange(N) // T
    dst = e // EPC
    el = e % EPC
    local_t = np.arange(N) % T

    def pack(mask):
        rank = np.zeros(N, np.int64)
        cnt = np.zeros((NCORES, NCORES), np.int64)
        for n in np.nonzero(mask)[0]:
            rank[n] = cnt[src[n], dst[n]]
            cnt[src[n], dst[n]] += 1
        return rank, _r16(max(cnt.max(), 1))

    # ---- dispatch: 4 chunks by local token quarter
    dq = [pack(keep & (local_t // QT == q)) for q in range(NQ)]
    Kq = tuple(k for _, k in dq)
    OFF1 = np.concatenate([[0], np.cumsum([NCORES * k for k in Kq])])
    R1 = int(OFF1[-1])                   # trash row in recv1

    srcQ = np.zeros(N, np.int64)
    for q in range(NQ):
        rank, K = dq[q]
        inq = local_t // QT == q
        srcQ = np.where(inq & keep, dst * K + rank, srcQ)
        srcQ = np.where(inq & ~keep, NCORES * K, srcQ)

    recv_row = np.full((NCORES, EPC * C), R1, np.int64)
    for q in range(NQ):
        rank, K = dq[q]
        for n in np.nonzero(keep & (local_t // QT == q))[0]:
            recv_row[dst[n], el[n] * C + pos[n]] = \
                OFF1[q] + src[n] * K + rank[n]

    # ---- combine: 4 chunks by (expert parity, capacity half)
    order = [(eli, ck) for eli in range(EPC) for ck in range(NCK)]
    comb = {c: pack(keep & (el == c[0]) & (pos // CH == c[1]))
            for c in order}
    K2 = tuple(comb[c][1] for c in order)
    OFF2 = {}
    acc = 0
    for c, k in zip(order, K2):
        OFF2[c] = acc
        acc += NCORES * k
    R2 = acc                             # trash row in recv2

    sl2 = np.zeros((NCORES, EPC, C), np.int64)
    for c, k in zip(order, K2):
        sl2[:, c[0], c[1] * CH:(c[1] + 1) * CH] = NCORES * k
    ygather = np.full(N, R2, np.int64)
    for n in np.nonzero(keep)[0]:
        c = (el[n], pos[n] // CH)
        rank, k = comb[c]
        sl2[dst[n], el[n], pos[n]] = src[n] * k + rank[n]
        ygather[n] = OFF2[c] + dst[n] * k + rank[n]

    per_core = []
    for k in range(NCORES):
        tok = slice(k * T, (k + 1) * T)
        sq = srcQ[tok]
        # dispatch scatter idx: position j = a*128+p <-> token q*512+4p+a
        # (xk tile [p, a] holds token 4p+a within the chunk)
        sd = np.concatenate(
            [_wrap16(sq[q * QT:(q + 1) * QT].reshape(P, 4).T.reshape(QT))
             for q in range(NQ)], axis=1)
        # combine scatter idx: position j = tt*128+p <-> slot ck*512+j
        sc = np.concatenate(
            [_wrap16(sl2[k, c[0], c[1] * CH:(c[1] + 1) * CH])
             for c in order], axis=1)
        per_core.append({
            "sd16": sd,
            "slot16": _wrap16(recv_row[k]),
            "sc16": sc,
            "ygather16": _wrap16(ygather[tok]),
        })
    return (Kq, K2), per_core


def _build_nc(cfg):
    Kq, K2 = cfg
    OFF1 = np.concatenate([[0], np.cumsum([NCORES * k for k in Kq])])
    R1 = int(OFF1[-1])
    order = [(eli, ck) for eli in range(EPC) for ck in range(NCK)]
    OFF2 = {}
    acc = 0
    for c, k in zip(order, K2):
        OFF2[c] = acc
        acc += NCORES * k
    R2 = acc
    K2d = dict(zip(order, K2))

    nc = bacc.Bacc("TRN2", target_bir_lowering=False, debug=False,
                   num_devices=NCORES)

    x_ext = nc.dram_tensor("x_ext", [T + 1, D], BF16, kind="ExternalInput")
    maa_k = nc.dram_tensor("maa_k", [1, D], BF16, kind="ExternalInput")
    maa_r = nc.dram_tensor("maa_r", [1, D], BF16, kind="ExternalInput")
    wrt = nc.dram_tensor("wrt", [D, D], BF16, kind="ExternalInput")
    wk = nc.dram_tensor("wk", [EPC, D, F], BF16, kind="ExternalInput")
    wv = nc.dram_tensor("wv", [EPC, F, D], BF16, kind="ExternalInput")
    sd16 = nc.dram_tensor("sd16", [P, T // 16], I16, kind="ExternalInput")
    slot16 = nc.dram_tensor("slot16", [P, EPC * C // 16], I16,
                            kind="ExternalInput")
    sc16 = nc.dram_tensor("sc16", [P, EPC * C // 16], I16,
                          kind="ExternalInput")
    ygather16 = nc.dram_tensor("ygather16", [P, T // 16], I16,
                               kind="ExternalInput")
    iota16 = nc.dram_tensor("iota16", [P, QT // 16], I16,
                            kind="ExternalInput")
    out = nc.dram_tensor("out", [T, D], F32, kind="ExternalOutput")

    DC = D // P          # 8
    FC = F // P          # 16
    rg = [list(range(NCORES))]

    with tile.TileContext(nc) as tc:
        with (
            tc.tile_pool(name="dram", bufs=1, space="DRAM") as dram,
            tc.tile_pool(name="misc", bufs=1) as misc,
            tc.tile_pool(name="pwk", bufs=1) as pwk,
            tc.tile_pool(name="pwv", bufs=1) as pwv,
            tc.tile_pool(name="psh", bufs=2, space="PSUM") as psh,
            tc.tile_pool(name="psy", bufs=2, space="PSUM") as psy,
        ):
            disp = [dram.tile([NCORES * Kq[q] + 1, D], BF16, name=f"disp{q}")
                    for q in range(NQ)]
            recv1 = dram.tile([R1 + 1, D], BF16)
            a2 = {c: dram.tile([NCORES * K2d[c] + 1, D], BF16,
                               name=f"a2_{c[0]}_{c[1]}")
                  for c in order}
            recv2 = dram.tile([R2 + 1, D], BF16)
            r_buf = dram.tile([T, D], BF16)

            zrow = misc.tile([1, D], BF16)
            nc.vector.memzero(zrow[:])
            nc.scalar.dma_start(out=recv1[R1:R1 + 1, :], in_=zrow[:])
            nc.scalar.dma_start(out=recv2[R2:R2 + 1, :], in_=zrow[:])

            # dispatch index on sync (needed first), the rest on scalar
            sD = misc.tile([P, T // 16], I16)
            nc.sync.dma_start(out=sD[:], in_=sd16[:])
            sl16 = misc.tile([P, EPC * C // 16], I16)
            nc.scalar.dma_start(out=sl16[:], in_=slot16[:])
            sC = misc.tile([P, EPC * C // 16], I16)
            nc.scalar.dma_start(out=sC[:], in_=sc16[:])
            yg16 = misc.tile([P, T // 16], I16)
            nc.scalar.dma_start(out=yg16[:], in_=ygather16[:])
            io16 = misc.tile([P, QT // 16], I16)
            nc.scalar.dma_start(out=io16[:], in_=iota16[:])

            # zero-fill the scatter-add target regions (pad rows are never
            # gathered on the recv side, but written rows need 0 for +=)
            ZR = 4
            zb = misc.tile([P, ZR, D], BF16)
            nc.vector.memzero(zb[:])

            def zero_fill(buf, rows):
                for off in range(0, rows, ZR * P):
                    n = min(ZR * P, rows - off)
                    nc.scalar.dma_start(
                        out=buf[off:off + n, :].rearrange(
                            "(a p) d -> p a d", p=P),
                        in_=zb[:, 0:n // P, :])



            wk_t = [pwk.tile([P, DC, F], BF16, tag="wk", name=f"wk_t{i}")
                    for i in range(EPC)]
            wv_t = [pwv.tile([P, FC, D], BF16, tag="wv", name=f"wv_t{i}")
                    for i in range(EPC)]

            # ---- phase A (token shift) + receptance, 4 chunks of 512
            with (
                tc.tile_pool(name="pa", bufs=2) as pa,
                tc.tile_pool(name="pdx", bufs=3) as pdx,
                tc.tile_pool(name="pam", bufs=1) as pam,
                tc.tile_pool(name="prx", bufs=2) as prx,
                tc.tile_pool(name="prs", bufs=1) as prs,
                tc.tile_pool(name="psr", bufs=1, space="PSUM") as psr,
            ):
                maakb = pam.tile([P, D], BF16)
                maarb = pam.tile([P, D], BF16)
                nc.scalar.dma_start(out=maakb[:],
                                    in_=maa_k[:].to_broadcast([P, D]))
                nc.scalar.dma_start(out=maarb[:],
                                    in_=maa_r[:].to_broadcast([P, D]))
                # wrt ships pre-shuffled so this is partition-contiguous
                wrt_sb = pam.tile([P, DC, D], BF16)
                nc.scalar.dma_start(out=wrt_sb[:],
                                    in_=wrt.rearrange("(p c) e -> p c e",
                                                      p=P))
                # zero-fills go on the scalar queue AFTER maa/wrt (those
                # gate the first DVE ops / receptance matmuls)
                for q in range(NQ):
                    zero_fill(disp[q], NCORES * Kq[q])

                pend_trig = None
                for q in range(NQ):
                    xq = pa.tile([P, 4, D], BF16, tag="xq")
                    nc.sync.dma_start(
                        out=xq[:],
                        in_=x_ext[1 + q * QT:1 + (q + 1) * QT, :].rearrange(
                            "(p a) d -> p a d", p=P))
                    # xprev strip for a=0: tokens 4p-1 = x_ext rows q*QT+4p
                    xp0 = pa.tile([P, 1, D], BF16, tag="xp0")
                    nc.sync.dma_start(
                        out=xp0[:],
                        in_=x_ext[q * QT:(q + 1) * QT, :].rearrange(
                            "(p a) d -> p a d", p=P)[:, 0:1, :])
                    # xprev for a=1..3 is xq shifted by one within the tile
                    dx = pdx.tile([P, 4, D], BF16, tag="dx")
                    nc.vector.tensor_sub(out=dx[:, 0:1, :], in0=xp0[:],
                                         in1=xq[:, 0:1, :])
                    nc.vector.tensor_sub(out=dx[:, 1:4, :],
                                         in0=xq[:, 0:3, :], in1=xq[:, 1:4, :])
                    tmp = pa.tile([P, 4, D], BF16, tag="tmp")
                    for n in range(4):
                        nc.vector.tensor_mul(out=tmp[:, n, :],
                                             in0=dx[:, n, :], in1=maakb[:])
                    xk = pa.tile([P, 4, D], BF16, tag="xk")
                    nc.vector.tensor_add(out=xk[:], in0=tmp[:], in1=xq[:])
                    nc.gpsimd.dma_scatter_add(
                        out_ap=disp[q][:], in_ap=xk[:],
                        idxs_ap=sD[:, q * 32:(q + 1) * 32],
                        num_idxs=QT, num_idxs_reg=QT, elem_size=D)
                    # the trigger for chunk q-1 goes here: its wait on the
                    # q-1 scatter completion has mostly elapsed by now, so
                    # it doesn't head-of-line-block the gpsimd queue
                    if pend_trig is not None:
                        pend_trig()
                    qq = q

                    def _trig(qq=qq):
                        return nc.gpsimd.collective_compute(
                            "AllToAll", mybir.AluOpType.bypass,
                            replica_groups=rg,
                            ins=[disp[qq][0:NCORES * Kq[qq], :]],
                            outs=[recv1[int(OFF1[qq]):int(OFF1[qq + 1]), :]])
                    pend_trig = _trig

                    # xr built in-place in dx (dx is dead after this)
                    for n in range(4):
                        nc.vector.tensor_mul(out=dx[:, n, :],
                                             in0=dx[:, n, :], in1=maarb[:])
                    nc.vector.tensor_add(out=dx[:], in0=dx[:], in1=xq[:])

                    # receptance for this chunk (PE soaks while A2A flies):
                    # SBUF-source transposing gather straight from the xr
                    # tile -- no DRAM round trip. Layout maps via
                    # tokens_per_rank=128: idx value = a*128+p.
                    xrT = prx.tile([P, DC, QT], BF16, tag="xrT")
                    nc.gpsimd.dma_gather(
                        out_ap=xrT[:], in_ap=dx[:],
                        idxs_ap=io16[:],
                        num_idxs=QT, num_idxs_reg=QT, elem_size=D,
                        transpose=True,
                        sbuf_tokens_per_rank=P,
                        sbuf_free_dim_per_rank=D * 2)
                    if q == NQ - 1:
                        last_trig = pend_trig()
                        pend_trig = None
                    rsb = prs.tile([P, 4, D], BF16, tag="rsb")
                    for tt in range(4):
                        pr0 = psr.tile([P, 512], F32, space="PSUM", tag="pr0")
                        pr1 = psr.tile([P, 512], F32, space="PSUM", tag="pr1")
                        for dc in range(DC):
                            nc.tensor.matmul(
                                out=pr0[:],
                                lhsT=xrT[:, dc, tt * P:(tt + 1) * P],
                                rhs=wrt_sb[:, dc, 0:512],
                                start=(dc == 0), stop=(dc == DC - 1))
                            nc.tensor.matmul(
                                out=pr1[:],
                                lhsT=xrT[:, dc, tt * P:(tt + 1) * P],
                                rhs=wrt_sb[:, dc, 512:1024],
                                start=(dc == 0), stop=(dc == DC - 1))
                        nc.scalar.activation(out=rsb[:, tt, 0:512],
                                             in_=pr0[:], func=AF.Sigmoid)
                        nc.scalar.activation(out=rsb[:, tt, 512:1024],
                                             in_=pr1[:], func=AF.Sigmoid)
                    nc.scalar.dma_start(
                        out=r_buf[q * QT:(q + 1) * QT, :].rearrange(
                            "(a p) d -> p a d", p=P),
                        in_=rsb[:])

            # expert-0 weight loads: held back behind the last dispatch
            # trigger so they don't steal HBM from the phase-A window
            wl0 = nc.sync.dma_start(
                out=wk_t[0][:], in_=wk[0].rearrange("(p c) f -> p c f", p=P))
            add_dep_helper(wl0.ins, last_trig.ins,
                           reason="keep wk0 load out of the phase-A window")
            nc.sync.dma_start(out=wv_t[0][:],
                              in_=wv[0].rearrange("(p c) f -> p c f", p=P))

            # zero-fill combine scatter targets (first use is mid-FFN)
            for c in order:
                zero_fill(a2[c], NCORES * K2d[c])

            # ---------------- phase C: expert FFNs
            with (
                tc.tile_pool(name="pfx", bufs=2) as pfx,
                tc.tile_pool(name="pfh", bufs=1) as pfh,
                tc.tile_pool(name="pfr", bufs=2) as pfr,
                tc.tile_pool(name="pfy", bufs=2) as pfy,
            ):
                for elp in range(EPC):
                    if elp > 0:
                        nc.sync.dma_start(
                            out=wk_t[elp][:],
                            in_=wk[elp].rearrange("(p c) f -> p c f", p=P))
                        nc.sync.dma_start(
                            out=wv_t[elp][:],
                            in_=wv[elp].rearrange("(p c) f -> p c f", p=P))
                    wk_sb, wv_sb = wk_t[elp], wv_t[elp]
                    for ck in range(NCK):
                        XT = pfx.tile([P, DC, 512], BF16, tag="XT")
                        col0 = (elp * C + ck * CH) // 16
                        nc.gpsimd.dma_gather(
                            out_ap=XT[:], in_ap=recv1[:],
                            idxs_ap=sl16[:, col0:col0 + 32],
                            num_idxs=512, num_idxs_reg=512, elem_size=D,
                            transpose=True)
                        ht = pfh.tile([P, FC, 512], BF16, tag="ht")
                        for ft in range(FC):
                            ph = psh.tile([P, 512], F32, space="PSUM",
                                          tag="ph")
                            for dc in range(DC):
                                nc.tensor.matmul(
                                    out=ph[:],
                                    lhsT=wk_sb[:, dc, ft * P:(ft + 1) * P],
                                    rhs=XT[:, dc, :],
                                    start=(dc == 0), stop=(dc == DC - 1))
                            hr = pfr.tile([P, 512], BF16, tag="hr")
                            nc.scalar.activation(out=hr[:], in_=ph[:],
                                                 func=AF.Relu)
                            nc.vector.tensor_mul(out=ht[:, ft, :], in0=hr[:],
                                                 in1=hr[:])
                        ysb = pfy.tile([P, 4, D], BF16, tag="ysb")
                        for tt in range(4):
                            py0 = psy.tile([P, 512], F32, space="PSUM",
                                           tag="py0")
                            py1 = psy.tile([P, 512], F32, space="PSUM",
                                           tag="py1")
                            for fc in range(FC):
                                nc.tensor.matmul(
                                    out=py0[:],
                                    lhsT=ht[:, fc, tt * P:(tt + 1) * P],
                                    rhs=wv_sb[:, fc, 0:512],
                                    start=(fc == 0), stop=(fc == FC - 1))
                                nc.tensor.matmul(
                                    out=py1[:],
                                    lhsT=ht[:, fc, tt * P:(tt + 1) * P],
                                    rhs=wv_sb[:, fc, 512:1024],
                                    start=(fc == 0), stop=(fc == FC - 1))
                            nc.scalar.activation(out=ysb[:, tt, 0:512],
                                                 in_=py0[:], func=AF.Copy)
                            nc.scalar.activation(out=ysb[:, tt, 512:1024],
                                                 in_=py1[:], func=AF.Copy)
                        cc = (elp, ck)
                        scol = (elp * NCK + ck) * 32
                        nc.gpsimd.dma_scatter_add(
                            out_ap=a2[cc][:], in_ap=ysb[:],
                            idxs_ap=sC[:, scol:scol + 32],
                            num_idxs=CH, num_idxs_reg=CH, elem_size=D)
                        nc.gpsimd.collective_compute(
                            "AllToAll", mybir.AluOpType.bypass,
                            replica_groups=rg,
                            ins=[a2[cc][0:NCORES * K2d[cc], :]],
                            outs=[recv2[OFF2[cc]:OFF2[cc] + NCORES * K2d[cc],
                                        :]])

            # ---------------- phase D: gather own rows, multiply by r
            with (
                tc.tile_pool(name="pdy", bufs=4) as pdy,
                tc.tile_pool(name="pdr", bufs=4) as pdr,
                tc.tile_pool(name="pd", bufs=2) as pd,
            ):
                rws = []
                for ck in range(T // 512):
                    rw = pdr.tile([P, 4, D], BF16, tag="rw")
                    nc.sync.dma_start(
                        out=rw[:],
                        in_=r_buf[ck * 512:(ck + 1) * 512, :].rearrange(
                            "(a p) d -> p a d", p=P))
                    rws.append(rw)
                ygs = []
                for ck in range(T // 512):
                    yg = pdy.tile([P, 4, D], BF16, tag="yg")
                    nc.gpsimd.dma_gather(
                        out_ap=yg[:], in_ap=recv2[:],
                        idxs_ap=yg16[:, ck * 32:(ck + 1) * 32],
                        num_idxs=512, num_idxs_reg=512, elem_size=D,
                        transpose=False)
                    ygs.append(yg)
                for ck in range(T // 512):
                    yo = pd.tile([P, 4, D], F32, tag="yo")
                    nc.vector.tensor_mul(out=yo[:], in0=ygs[ck][:],
                                         in1=rws[ck][:])
                    nc.scalar.dma_start(
                        out=out[ck * 512:(ck + 1) * 512, :].rearrange(
                            "(a p) d -> p a d", p=P),
                        in_=yo[:])

    nc.finalize()
    return nc


def _shuffle_rows(w, nchunks):
    """[R, ...] -> row p*nchunks+c holds original row c*128+p."""
    r = w.shape[0]
    assert r == nchunks * P
    return np.ascontiguousarray(
        w.reshape(nchunks, P, -1).transpose(1, 0, 2).reshape(w.shape))


def _prepare_inputs(x, token_ids, shift_state, time_maa_k, time_maa_r,
                    w_recept, w_key, w_value):
    cfg, idxs = _build_indices(token_ids)
    x = np.asarray(x, np.float32)
    shift = np.asarray(shift_state, np.float32)
    wrt = _shuffle_rows(
        np.ascontiguousarray(np.asarray(w_recept, np.float32).T), D // P
    ).astype(nbf16)
    wkb = np.asarray(w_key, np.float32).astype(nbf16)
    wkb = np.stack([_shuffle_rows(wkb[e], D // P) for e in range(E)])
    wvb = np.asarray(w_value, np.float32).astype(nbf16)
    wvb = np.stack([_shuffle_rows(wvb[e], F // P) for e in range(E)])
    mk = np.asarray(time_maa_k, np.float32)[None, :].astype(nbf16)
    mr = np.asarray(time_maa_r, np.float32)[None, :].astype(nbf16)
    # SBUF-source gather idx: output position j (= token q*512+j) reads
    # rank j//4 (partition), row j%4 -> idx value = (j%4)*128 + j//4
    j = np.arange(QT, dtype=np.int16)
    iota = _wrap16((j % 4) * P + j // 4)

    in_maps = []
    for k in range(NCORES):
        x_ext = np.concatenate([shift[k:k + 1], x[k]], axis=0).astype(nbf16)
        in_maps.append({
            "x_ext": np.ascontiguousarray(x_ext),
            "maa_k": mk, "maa_r": mr, "wrt": wrt,
            "wk": np.ascontiguousarray(wkb[EPC * k:EPC * (k + 1)]),
            "wv": np.ascontiguousarray(wvb[EPC * k:EPC * (k + 1)]),
            "iota16": iota,
            **idxs[k],
        })
    return cfg, in_maps


def kernel(x, token_ids, shift_state, time_maa_k, time_maa_r,
           w_recept, w_key, w_value, _trace=False):
    cfg, in_maps = _prepare_inputs(x, token_ids, shift_state, time_maa_k,
                                   time_maa_r, w_recept, w_key, w_value)
    if cfg not in _CACHE:
        _CACHE[cfg] = _build_nc(cfg)
    nc = _CACHE[cfg]
    res = run_bass_kernel_spmd(nc, in_maps, core_ids=list(range(NCORES)),
                               trace=_trace)
    kernel.last_result = res
    y = np.stack([res.results[k]["out"] for k in range(NCORES)], axis=0)
    return y.astype(np.float32)
